# revision 32
# baseline (speedup 1.0000x reference)
"""Trainium2 Bass kernel for nn_AutoGraphModel (GNN message passing).

Strategy (8 NeuronCores, SPMD):
  - Nodes are range-sharded across cores in 128-aligned blocks; edges are
    sharded by destination node and sorted by dst on the host (layout only).
  - Per core, dst-range edges are processed in 128-edge tiles grouped into
    128-node "windows"; segment-sum (scatter-add) is done race-free on the
    TensorEngine via one-hot selection matmuls accumulating in PSUM.
  - gconv1 gathers 2-float node features via indirect DMA (8B rows);
    gconv2 gathers 128-float rows of the (replicated) h1 table.
  - h1 is computed sharded and replicated with an AllGather; graph pooling
    partials are combined with an AllReduce (the "psum" boundary).
  - The tiny transformer/MLP head runs replicated on every core.

Host-side work is strictly integer layout/sharding: sorting edge lists,
CSR rowptr construction, padding, and concatenating parameter arrays.
All floating-point math happens on device.
"""

import os
import numpy as np

import concourse.bacc as bacc
import concourse.bass as bass
import concourse.mybir as mybir
import concourse.tile as tile
from concourse.masks import make_identity

FP32 = mybir.dt.float32
FP16 = mybir.dt.float16
I32 = mybir.dt.int32
AF = mybir.ActivationFunctionType
ALU = mybir.AluOpType

NCORES = 8
B = 8
DM, NH, HD, HID, FF = 32, 4, 8, 128, 256
TOK = B * 12  # 96 tokens


class Cfg:
    def __init__(self, n_true, n_edges, KW):
        self.NT = n_true
        self.E = n_edges
        per = -(-n_true // NCORES)            # ceil
        self.RANGE = -(-per // 128) * 128     # 128-aligned nodes per core
        self.W = self.RANGE // 128            # windows per core
        self.NPAD = NCORES * self.RANGE
        self.XN = self.NPAD // 128
        self.KW = list(KW)                    # tiles per window (max over cores)
        self.TSTART = np.concatenate([[0], np.cumsum(self.KW)]).astype(int)
        self.T = int(self.TSTART[-1])         # edge tiles per core


# ----------------------------------------------------------------------------
# Host-side integer preprocessing (layout / sharding only)
# ----------------------------------------------------------------------------

def host_prep(edge_src, edge_dst, node2graph, n_true, n_edges):
    es = np.asarray(edge_src, dtype=np.int64)
    ed = np.asarray(edge_dst, dtype=np.int64)
    n2g = np.asarray(node2graph, dtype=np.int32)
    N = n_true

    order = np.argsort(ed, kind="stable")
    ed_s = ed[order]
    es_s = es[order].astype(np.int32)

    cnt_dst = np.bincount(ed, minlength=N)
    cnt_src = np.bincount(es, minlength=N)
    rp_dst = np.concatenate([[0], np.cumsum(cnt_dst)]).astype(np.int32)
    rp_src = np.concatenate([[0], np.cumsum(cnt_src)]).astype(np.int32)

    # First pass: per-window tile counts (max over cores)
    cfg0 = Cfg(n_true, n_edges, [1])
    RANGE, W, NPAD, XN = cfg0.RANGE, cfg0.W, cfg0.NPAD, cfg0.XN

    KW = np.ones(W, dtype=np.int64)
    core_edges = []
    for c in range(NCORES):
        base = c * RANGE
        lo = np.searchsorted(ed_s, base, side="left")
        hi = np.searchsorted(ed_s, base + RANGE, side="left")
        dl_all = (ed_s[lo:hi] - base).astype(np.int32)
        src_all = es_s[lo:hi]
        win = dl_all >> 7
        wcnt = np.bincount(win, minlength=W)
        KW = np.maximum(KW, -(-wcnt // 128))
        core_edges.append((dl_all, src_all, win, wcnt))

    cfg = Cfg(n_true, n_edges, KW)
    T = cfg.T
    TSTART = cfg.TSTART

    def layout_128(arr_1d, npad, fill):
        out = np.full(npad, fill, dtype=np.int32)
        out[: arr_1d.shape[0]] = arr_1d
        return np.ascontiguousarray(out.reshape(-1, 128).T)  # [128, npad/128]

    rsD = layout_128(rp_dst[:N], NPAD, rp_dst[N])
    reD = layout_128(rp_dst[1 : N + 1], NPAD, rp_dst[N])
    rsS = layout_128(rp_src[:N], NPAD, rp_src[N])
    reS = layout_128(rp_src[1 : N + 1], NPAD, rp_src[N])

    per_core = []
    for c in range(NCORES):
        dl_all, src_all, win, wcnt = core_edges[c]
        ne = dl_all.shape[0]
        wstart = np.concatenate([[0], np.cumsum(wcnt)])
        j_in_w = np.arange(ne, dtype=np.int64) - wstart[win]
        pos = TSTART[win] * 128 + j_in_w
        esrc_flat = np.zeros(T * 128, dtype=np.int32)
        edl_flat = np.full(T * 128, -1, dtype=np.int32)
        esrc_flat[pos] = src_all
        edl_flat[pos] = dl_all & 127
        esrc_sb = np.ascontiguousarray(esrc_flat.reshape(T, 128).T)  # [128,T]
        edl_sb = np.ascontiguousarray(edl_flat.reshape(T, 128).T)

        base = c * RANGE

        def own_layout(arr_np1, fill):
            seg = np.full(RANGE, fill, dtype=np.int32)
            m = max(0, min(RANGE, N - base))
            seg[:m] = arr_np1[base : base + m]
            return np.ascontiguousarray(seg.reshape(W, 128).T)  # [128, W]

        rsD_o = own_layout(rp_dst[:N], rp_dst[N])
        reD_o = own_layout(rp_dst[1 : N + 1], rp_dst[N])
        rsS_o = own_layout(rp_src[:N], rp_src[N])
        reS_o = own_layout(rp_src[1 : N + 1], rp_src[N])
        n2g_o = own_layout(n2g, -1)
        per_core.append(
            dict(esrc=esrc_sb, edl=edl_sb, rsD_o=rsD_o, reD_o=reD_o,
                 rsS_o=rsS_o, reS_o=reS_o, n2g_o=n2g_o)
        )

    gid = np.ascontiguousarray((np.arange(TOK, dtype=np.int32) // 12)[None, :])
    rep = dict(rsD=rsD, reD=reD, rsS=rsS, reS=reS, gid=gid)
    return cfg, rep, per_core


def host_params(params):
    """Concatenate parameter arrays into device layouts (pure memcpy)."""
    def A(x):
        return np.ascontiguousarray(np.asarray(x, dtype=np.float32))

    def wb(p):  # [K+1, M] = [W; b]
        return np.concatenate([A(p["W"]), A(p["b"])[None, :]], axis=0)

    out = {}
    out["gc1_w"] = A(params["gc1"]["W"])                    # [2, 128]
    out["gc1_b"] = A(params["gc1"]["b"])[None, :]           # [1, 128]
    out["gc2_w"] = A(params["gc2"]["W"])                    # [128, 128]
    out["gc2_b"] = A(params["gc2"]["b"])[None, :]           # [1, 128]
    out["glin_w"] = A(params["graph_lin"]["W"])             # [128, 128]
    out["glin_b"] = A(params["graph_lin"]["b"])[None, :]
    out["algo1_wb"] = wb(params["algo1"])                   # [97, 128]
    out["algo2_w"] = A(params["algo2"]["W"])
    out["algo2_b"] = A(params["algo2"]["b"])[None, :]
    out["sched1_w"] = A(params["sched1"]["W"])
    out["sched1_b"] = A(params["sched1"]["b"])[None, :]
    out["sched2_w"] = A(params["sched2"]["W"])
    out["sched2_b"] = A(params["sched2"]["b"])[None, :]
    out["A_algo"] = np.concatenate(
        [A(params["msg_create"])[:2], A(params["msg_reduce"])[:2],
         A(params["compute_mode"])[:2]], axis=1)           # [2, 96]
    out["A_sched"] = np.concatenate(
        [A(params["direction"])[:2], A(params["parallel"])[:2],
         A(params["frontier"])[:2], A(params["ssg"])[:2]], axis=1)  # [2,128]
    for l, lp in enumerate(params["tf"]):
        out[f"tf{l}_q"] = wb(lp["q"])       # [33, 32]
        out[f"tf{l}_k"] = wb(lp["k"])
        out[f"tf{l}_v"] = wb(lp["v"])
        out[f"tf{l}_o"] = wb(lp["o"])
        out[f"tf{l}_ff1"] = wb(lp["ff1"])   # [33, 256]
        out[f"tf{l}_ff2w"] = np.ascontiguousarray(
            A(lp["ff2"]["W"]).reshape(2, 128, DM).transpose(1, 0, 2))
        out[f"tf{l}_ff2b"] = A(lp["ff2"]["b"])[None, :]     # [1, 32]
        out[f"tf{l}_ln1g"] = np.broadcast_to(A(lp["ln1_g"])[None, :], (TOK, DM)).copy()
        out[f"tf{l}_ln1b"] = np.broadcast_to(A(lp["ln1_b"])[None, :], (TOK, DM)).copy()
        out[f"tf{l}_ln2g"] = np.broadcast_to(A(lp["ln2_g"])[None, :], (TOK, DM)).copy()
        out[f"tf{l}_ln2b"] = np.broadcast_to(A(lp["ln2_b"])[None, :], (TOK, DM)).copy()
    out["mlp1_w"] = np.ascontiguousarray(
        A(params["mlp1"]["W"]).reshape(3, 128, HID).transpose(1, 0, 2))
    out["mlp1_b"] = A(params["mlp1"]["b"])[None, :]
    out["mlp2_w"] = A(params["mlp2"]["W"])   # [128, 64]
    out["mlp2_b"] = A(params["mlp2"]["b"])[None, :]
    out["mlp3_wb"] = wb(params["mlp3"])      # [65, 1]
    return out


# ----------------------------------------------------------------------------
# Device program
# ----------------------------------------------------------------------------

PARAM_SHAPES = None  # filled by build_program


def build_program(cfg, edge_fp16=True):
    """Builds the SPMD Bass program. Returns (nc, input_specs) where
    input_specs maps tensor-name -> (shape, np dtype)."""
    W, T, XN, NPAD, RANGE = cfg.W, cfg.T, cfg.XN, cfg.NPAD, cfg.RANGE
    KW, TSTART = cfg.KW, cfg.TSTART
    EDT = FP16 if edge_fp16 else FP32
    EDT_np = np.float16 if edge_fp16 else np.float32

    nc = bacc.Bacc("TRN2", target_bir_lowering=False, debug=False,
                   num_devices=NCORES)

    ins = {}

    def di(name, shape, dt):
        h = nc.dram_tensor(name, list(shape), dt, kind="ExternalInput")
        ins[name] = h
        return h

    # int inputs
    d_rsD = di("rsD", (128, XN), I32)
    d_reD = di("reD", (128, XN), I32)
    d_rsS = di("rsS", (128, XN), I32)
    d_reS = di("reS", (128, XN), I32)
    d_gid = di("gid", (1, TOK), I32)
    d_algo = di("algo", (B, 3), I32)
    d_sched = di("sched", (B, 4), I32)
    d_esrc = di("esrc", (128, T), I32)
    d_edl = di("edl", (128, T), I32)
    d_rsD_o = di("rsD_o", (128, W), I32)
    d_reD_o = di("reD_o", (128, W), I32)
    d_rsS_o = di("rsS_o", (128, W), I32)
    d_reS_o = di("reS_o", (128, W), I32)
    d_n2g_o = di("n2g_o", (128, W), I32)

    # parameter inputs
    pshapes = {
        "gc1_w": (2, HID), "gc1_b": (1, HID),
        "gc2_w": (HID, HID), "gc2_b": (1, HID),
        "glin_w": (HID, HID), "glin_b": (1, HID),
        "algo1_wb": (97, HID), "algo2_w": (HID, HID), "algo2_b": (1, HID),
        "sched1_w": (HID, HID), "sched1_b": (1, HID),
        "sched2_w": (HID, HID), "sched2_b": (1, HID),
        "A_algo": (2, 96), "A_sched": (2, HID),
        "mlp1_w": (HID, 3, HID), "mlp1_b": (1, HID),
        "mlp2_w": (HID, 64), "mlp2_b": (1, 64), "mlp3_wb": (65, 1),
    }
    for l in range(4):
        pshapes[f"tf{l}_q"] = (33, DM)
        pshapes[f"tf{l}_k"] = (33, DM)
        pshapes[f"tf{l}_v"] = (33, DM)
        pshapes[f"tf{l}_o"] = (33, DM)
        pshapes[f"tf{l}_ff1"] = (33, FF)
        pshapes[f"tf{l}_ff2w"] = (HID, 2, DM)
        pshapes[f"tf{l}_ff2b"] = (1, DM)
        for nm in ("ln1g", "ln1b", "ln2g", "ln2b"):
            pshapes[f"tf{l}_{nm}"] = (TOK, DM)
    dpar = {k: di(k, v, FP32) for k, v in pshapes.items()}

    # internal DRAM
    d_vtab = nc.dram_tensor("vtab", [NPAD, 2], FP32)
    d_h1own = nc.dram_tensor("h1own", [RANGE, HID], EDT)
    d_h1full = nc.dram_tensor("h1full", [NPAD, HID], EDT, addr_space="Shared")
    d_poolp = nc.dram_tensor("poolp", [B, HID + 1], FP32)
    d_poolf = nc.dram_tensor("poolf", [B, HID + 1], FP32, addr_space="Shared")
    d_out = nc.dram_tensor("out", [B, 1], FP32, kind="ExternalOutput")
    dbg = os.environ.get("GNN_DBG", "0") == "1"
    if dbg:
        d_dbg_h1 = nc.dram_tensor("dbg_h1", [RANGE, HID], FP32,
                                  kind="ExternalOutput")
        d_dbg_pool = nc.dram_tensor("dbg_pool", [B, HID + 1], FP32,
                                    kind="ExternalOutput")
        d_dbg_poolf = nc.dram_tensor("dbg_poolf", [B, HID + 1], FP32,
                                     kind="ExternalOutput")
        d_dbg_xcat = nc.dram_tensor("dbg_xcat", [B, 3 * HID], FP32,
                                    kind="ExternalOutput")
        d_dbg_vtab = nc.dram_tensor("dbg_vtab", [NPAD, 2], FP32,
                                    kind="ExternalOutput")

    rg = [list(range(NCORES))]

    with tile.TileContext(nc) as tc:
        with tc.tile_pool(name="const", bufs=1) as cpool:
            # --- constants / params in SBUF ---
            idm = cpool.tile([128, 128], FP32)
            make_identity(nc, idm[:])
            ones_row = cpool.tile([1, 128], FP32)
            nc.vector.memset(ones_row[:], 1.0)
            eps_col = cpool.tile([128, 1], FP32)
            nc.vector.memset(eps_col[:], 1e-5)
            iota_i = cpool.tile([128, 128], I32)
            nc.gpsimd.iota(iota_i[:], pattern=[[1, 128]], base=0,
                           channel_multiplier=0)
            iota_e = cpool.tile([128, 128], EDT)
            nc.vector.tensor_copy(iota_e[:], iota_i[:])
            iota8 = cpool.tile([128, 8], FP32)
            nc.vector.tensor_copy(iota8[:], iota_i[:, :8])
            # iota over partitions (value = partition index), for one-hots
            iop_i = cpool.tile([128, B], I32)
            nc.gpsimd.iota(iop_i[:], pattern=[[0, B]], base=0,
                           channel_multiplier=1)
            iop = cpool.tile([128, B], FP32)
            nc.vector.tensor_copy(iop[:], iop_i[:])

            par = {}
            for k, sh in pshapes.items():
                t_ = cpool.tile(list(sh), FP32, tag=f"p_{k}")
                nc.sync.dma_start(t_[:], dpar[k][:])
                par[k] = t_

            # token-graph mask [96, 96]
            gcol_i = cpool.tile([TOK, 1], I32)
            nc.sync.dma_start(gcol_i[:],
                              bass.AP(d_gid, 0, [[1, TOK], [1, 1]]))
            grow_i = cpool.tile([TOK, TOK], I32)
            nc.sync.dma_start(grow_i[:], bass.AP(d_gid, 0, [[0, TOK], [1, TOK]]))
            gmask = cpool.tile([TOK, TOK], FP32)
            nc.vector.tensor_tensor(out=gmask[:], in0=grow_i[:],
                                    in1=gcol_i[:].to_broadcast([TOK, TOK]),
                                    op=ALU.is_equal)

            # --- degrees, norms, v-table ---
            with tc.tile_pool(name="deg", bufs=1) as dp:
                _dgc = [0]
                def deg_f32(pool, rs, re, n):
                    _dgc[0] += 1
                    ti = pool.tile([128, n], I32, tag=f"deg_i{_dgc[0]}")
                    nc.vector.tensor_tensor(out=ti[:], in0=re[:], in1=rs[:],
                                            op=ALU.subtract)
                    tf = pool.tile([128, n], FP32, tag=f"deg_f{_dgc[0]}")
                    nc.vector.tensor_scalar(out=tf[:], in0=ti[:], scalar1=1.0,
                                            scalar2=None, op0=ALU.add)
                    return tf

                rsD_s = dp.tile([128, XN], I32, tag="rp1")
                reD_s = dp.tile([128, XN], I32, tag="rp2")
                rsS_s = dp.tile([128, XN], I32, tag="rp3")
                reS_s = dp.tile([128, XN], I32, tag="rp4")
                nc.sync.dma_start(rsD_s[:], d_rsD[:])
                nc.sync.dma_start(reD_s[:], d_reD[:])
                nc.sync.dma_start(rsS_s[:], d_rsS[:])
                nc.sync.dma_start(reS_s[:], d_reS[:])
                din = deg_f32(dp, rsD_s, reD_s, XN)
                dout = deg_f32(dp, rsS_s, reS_s, XN)
                tmp = dp.tile([128, XN], FP32)
                nc.scalar.activation(tmp[:], dout[:], AF.Sqrt)
                dno = dp.tile([128, XN], FP32)
                nc.vector.reciprocal(dno[:], tmp[:])
                vbuf = dp.tile([128, XN, 2], FP32)
                nc.vector.tensor_tensor(out=vbuf[:, :, 0], in0=dno[:],
                                        in1=din[:], op=ALU.mult)
                nc.vector.tensor_tensor(out=vbuf[:, :, 1], in0=dno[:],
                                        in1=dout[:], op=ALU.mult)
                nc.sync.dma_start(
                    d_vtab[:].rearrange("(x p) f -> p x f", p=128), vbuf[:])

            # --- own-range node data (persist through B and C) ---
            rsDo_s = cpool.tile([128, W], I32, tag="rpo1")
            reDo_s = cpool.tile([128, W], I32, tag="rpo2")
            rsSo_s = cpool.tile([128, W], I32, tag="rpo3")
            reSo_s = cpool.tile([128, W], I32, tag="rpo4")
            nc.sync.dma_start(rsDo_s[:], d_rsD_o[:])
            nc.sync.dma_start(reDo_s[:], d_reD_o[:])
            nc.sync.dma_start(rsSo_s[:], d_rsS_o[:])
            nc.sync.dma_start(reSo_s[:], d_reS_o[:])

            dinO_i = cpool.tile([128, W], I32)
            nc.vector.tensor_tensor(out=dinO_i[:], in0=reDo_s[:],
                                    in1=rsDo_s[:], op=ALU.subtract)
            dinO = cpool.tile([128, W], FP32)
            nc.vector.tensor_scalar(out=dinO[:], in0=dinO_i[:], scalar1=1.0,
                                    scalar2=None, op0=ALU.add)
            doutO_i = cpool.tile([128, W], I32)
            nc.vector.tensor_tensor(out=doutO_i[:], in0=reSo_s[:],
                                    in1=rsSo_s[:], op=ALU.subtract)
            doutO = cpool.tile([128, W], FP32)
            nc.vector.tensor_scalar(out=doutO[:], in0=doutO_i[:], scalar1=1.0,
                                    scalar2=None, op0=ALU.add)
            tmpO = cpool.tile([128, W], FP32, tag="tmpO")
            nc.scalar.activation(tmpO[:], doutO[:], AF.Sqrt)
            dnoO = cpool.tile([128, W], FP32)
            nc.vector.reciprocal(dnoO[:], tmpO[:])
            tmpO2 = cpool.tile([128, W], FP32, tag="tmpO")
            nc.scalar.activation(tmpO2[:], dinO[:], AF.Sqrt)
            dniO = cpool.tile([128, W], FP32)
            nc.vector.reciprocal(dniO[:], tmpO2[:])
            vO = cpool.tile([128, W, 2], FP32)
            nc.vector.tensor_tensor(out=vO[:, :, 0], in0=dnoO[:], in1=dinO[:],
                                    op=ALU.mult)
            nc.vector.tensor_tensor(out=vO[:, :, 1], in0=dnoO[:], in1=doutO[:],
                                    op=ALU.mult)
            n2gF = cpool.tile([128, W], FP32)

            # edge metadata
            esrc_sb = cpool.tile([128, T], I32)
            nc.sync.dma_start(esrc_sb[:], d_esrc[:])
            edl_sb = cpool.tile([128, T], I32)
            nc.sync.dma_start(edl_sb[:], d_edl[:])
            edl_e = cpool.tile([128, T], FP32)
            nc.vector.tensor_copy(edl_e[:], edl_sb[:])

            n2g_i = cpool.tile([128, W], I32)
            nc.sync.dma_start(n2g_i[:], d_n2g_o[:])
            nc.vector.tensor_copy(n2gF[:], n2g_i[:])

            h1own_sb = cpool.tile([128, W, HID], FP32)  # 50KB/part

            # ---------------- phase B: gconv1 ----------------
            with tc.tile_pool(name="pbv", bufs=16) as pbv, \
                 tc.tile_pool(name="pbs", bufs=4) as pbs, \
                 tc.tile_pool(name="ppB", bufs=2, space="PSUM") as ppB, \
                 tc.tile_pool(name="ppT", bufs=2, space="PSUM") as ppT, \
                 tc.tile_pool(name="ppZ", bufs=2, space="PSUM") as ppZ:
                for w in range(W):
                    ps = ppB.tile([128, 2], FP32)
                    kw = KW[w]
                    for k in range(kw):
                        t = int(TSTART[w]) + k
                        gv = pbv.tile([128, 2], FP32, tag="gv")
                        nc.gpsimd.indirect_dma_start(
                            out=gv[:], out_offset=None, in_=d_vtab[:],
                            in_offset=bass.IndirectOffsetOnAxis(
                                ap=esrc_sb[:, t : t + 1], axis=0))
                        S = pbs.tile([128, 128], FP32, tag="SB")
                        nc.vector.tensor_scalar(
                            out=S[:], in0=iota_e[:],
                            scalar1=edl_e[:, t : t + 1], scalar2=None,
                            op0=ALU.is_equal)
                        nc.tensor.matmul(
                            ps[:], lhsT=S[:], rhs=gv[:],
                            start=(k == 0), stop=(k == kw - 1))
                    # drain: (+ self loop) * dn_in
                    t1 = pbs.tile([128, 2], FP32, tag="dr1")
                    nc.vector.tensor_tensor(out=t1[:], in0=ps[:],
                                            in1=vO[:, w, :], op=ALU.add)
                    t2 = pbs.tile([128, 2], FP32, tag="dr2")
                    nc.vector.tensor_scalar(out=t2[:], in0=t1[:],
                                            scalar1=dniO[:, w : w + 1],
                                            scalar2=None, op0=ALU.mult)
                    pt = ppT.tile([2, 128], FP32)
                    nc.tensor.transpose(pt[:], t2[:], idm[:])
                    lz = pbs.tile([2, 128], FP32, tag="lz")
                    nc.vector.tensor_copy(lz[:], pt[:])
                    zp = ppZ.tile([128, HID], FP32)
                    nc.tensor.matmul(zp[:], lhsT=lz[:], rhs=par["gc1_w"][:],
                                     start=True, stop=False)
                    nc.tensor.matmul(zp[:], lhsT=ones_row[:],
                                     rhs=par["gc1_b"][:],
                                     start=False, stop=True)
                    rt = pbs.tile([128, HID], FP32, tag="rt")
                    nc.vector.tensor_scalar(out=rt[:], in0=zp[:], scalar1=0.0,
                                            scalar2=None, op0=ALU.max)
                    mt = pbs.tile([128, HID], FP32, tag="mt")
                    nc.vector.tensor_scalar(out=mt[:], in0=zp[:], scalar1=0.0,
                                            scalar2=None, op0=ALU.min)
                    et = pbs.tile([128, HID], FP32, tag="et")
                    nc.scalar.activation(et[:], mt[:], AF.Exp)
                    st = pbs.tile([128, HID], FP32, tag="st")
                    nc.vector.tensor_tensor(out=st[:], in0=rt[:], in1=et[:],
                                            op=ALU.add)
                    nc.vector.tensor_scalar(out=h1own_sb[:, w, :], in0=st[:],
                                            scalar1=-1.0,
                                            scalar2=dnoO[:, w : w + 1],
                                            op0=ALU.add, op1=ALU.mult)

            # h1own -> DRAM (cast to EDT) and AllGather
            nc.gpsimd.dma_start(
                out=d_h1own[:].rearrange("(w p) f -> p w f", p=128),
                in_=h1own_sb[:])
            nc.gpsimd.collective_compute(
                "AllGather", ALU.bypass, ins=[d_h1own[:]], outs=[d_h1full[:]],
                replica_groups=rg)

            # ---------------- phase C: gconv2 + pooling ----------------
            with tc.tile_pool(name="pcg", bufs=16) as pcg, \
                 tc.tile_pool(name="pcs", bufs=4) as pcs, \
                 tc.tile_pool(name="ppC", bufs=2, space="PSUM") as ppC, \
                 tc.tile_pool(name="ppT2", bufs=2, space="PSUM") as ppT2, \
                 tc.tile_pool(name="ppZ2", bufs=2, space="PSUM") as ppZ2, \
                 tc.tile_pool(name="ppPool", bufs=1, space="PSUM") as ppPool:
                poolp = ppPool.tile([B, HID + 1], FP32)
                for w in range(W):
                    ps = ppC.tile([128, HID], FP32)
                    kw = KW[w]
                    for k in range(kw):
                        t = int(TSTART[w]) + k
                        gh = pcg.tile([128, HID], FP32, tag="gh")
                        nc.gpsimd.indirect_dma_start(
                            out=gh[:], out_offset=None, in_=d_h1full[:],
                            in_offset=bass.IndirectOffsetOnAxis(
                                ap=esrc_sb[:, t : t + 1], axis=0))
                        S = pcs.tile([128, 128], FP32, tag="SC")
                        nc.vector.tensor_scalar(
                            out=S[:], in0=iota_e[:],
                            scalar1=edl_e[:, t : t + 1], scalar2=None,
                            op0=ALU.is_equal)
                        nc.tensor.matmul(
                            ps[:], lhsT=S[:], rhs=gh[:],
                            start=(k == 0), stop=(k == kw - 1))
                    # drain
                    a1 = pcs.tile([128, HID], FP32, tag="a1")
                    nc.vector.tensor_tensor(out=a1[:], in0=ps[:],
                                            in1=h1own_sb[:, w, :], op=ALU.add)
                    a2 = pcs.tile([128, HID], FP32, tag="a2")
                    nc.vector.tensor_scalar(out=a2[:], in0=a1[:],
                                            scalar1=dniO[:, w : w + 1],
                                            scalar2=None, op0=ALU.mult)
                    pt = ppT2.tile([128, HID], FP32)
                    nc.tensor.transpose(pt[:], a2[:], idm[:])
                    lz = pcs.tile([128, HID], FP32, tag="lz2")
                    nc.vector.tensor_copy(lz[:], pt[:])
                    zp = ppZ2.tile([128, HID], FP32)
                    nc.tensor.matmul(zp[:], lhsT=lz[:], rhs=par["gc2_w"][:],
                                     start=True, stop=False)
                    nc.tensor.matmul(zp[:], lhsT=ones_row[:],
                                     rhs=par["gc2_b"][:],
                                     start=False, stop=True)
                    rt = pcs.tile([128, HID], FP32, tag="rt2")
                    nc.vector.tensor_scalar(out=rt[:], in0=zp[:], scalar1=0.0,
                                            scalar2=None, op0=ALU.max)
                    mt = pcs.tile([128, HID], FP32, tag="mt2")
                    nc.vector.tensor_scalar(out=mt[:], in0=zp[:], scalar1=0.0,
                                            scalar2=None, op0=ALU.min)
                    et = pcs.tile([128, HID], FP32, tag="et2")
                    nc.scalar.activation(et[:], mt[:], AF.Exp)
                    h2t = pcs.tile([128, HID + 1], FP32, tag="h2t")
                    nc.vector.tensor_tensor(out=h2t[:, :HID], in0=rt[:],
                                            in1=et[:], op=ALU.add)
                    nc.vector.memset(h2t[:, HID : HID + 1], 1.0)
                    G = pcs.tile([128, B], FP32, tag="G")
                    nc.vector.tensor_scalar(out=G[:], in0=iota8[:],
                                            scalar1=n2gF[:, w : w + 1],
                                            scalar2=None, op0=ALU.is_equal)
                    nc.tensor.matmul(poolp[:], lhsT=G[:], rhs=h2t[:],
                                     start=(w == 0), stop=(w == W - 1),
                                     skip_group_check=True)

                pool_sb = pcs.tile([B, HID + 1], FP32, tag="poolsb")
                nc.vector.tensor_copy(pool_sb[:], poolp[:])
                nc.sync.dma_start(d_poolp[:], pool_sb[:])
                if dbg:
                    nc.sync.dma_start(d_dbg_pool[:], pool_sb[:])

            nc.gpsimd.collective_compute(
                "AllReduce", ALU.add, ins=[d_poolp[:]], outs=[d_poolf[:]],
                replica_groups=rg)

            # ---------------- tail (replicated on every core) --------------
            with tc.tile_pool(name="tl", bufs=2) as tl, \
                 tc.tile_pool(name="ppX", bufs=3, space="PSUM") as ppX, \
                 tc.tile_pool(name="ppY", bufs=3, space="PSUM") as ppY:

                def mm(out_ap, lhs_ap, rhs_ap, start=True, stop=True):
                    nc.tensor.matmul(out_ap, lhsT=lhs_ap, rhs=rhs_ap,
                                     start=start, stop=stop)

                def transpose_to(pool, in_ap, pdim, fdim, extra=0, tag=None):
                    """Return SBUF tile [fdim(+extra), pdim] holding in_ap^T
                    (+ optional ones row at index fdim)."""
                    pt = ppX.tile([fdim, pdim], FP32, tag="x")
                    nc.tensor.transpose(pt[:], in_ap, idm[:pdim, :pdim])
                    st = pool.tile([fdim + extra, pdim], FP32, tag=f"Ts{tag}")
                    nc.vector.tensor_copy(st[:fdim, :], pt[:])
                    if extra:
                        nc.vector.memset(st[fdim : fdim + 1, :], 1.0)
                    return st

                # pooled graph feature
                poolF = tl.tile([B, HID + 1], FP32)
                nc.sync.dma_start(poolF[:], d_poolf[:])
                cntm = tl.tile([B, 1], FP32)
                nc.vector.tensor_scalar(out=cntm[:],
                                        in0=poolF[:, HID : HID + 1],
                                        scalar1=1.0, scalar2=None, op0=ALU.max)
                rcnt = tl.tile([B, 1], FP32)
                nc.vector.reciprocal(rcnt[:], cntm[:])
                hgt = tl.tile([B, HID], FP32)
                nc.vector.tensor_scalar(out=hgt[:], in0=poolF[:, :HID],
                                        scalar1=poolF[:, HID : HID + 1],
                                        scalar2=None, op0=ALU.subtract)
                hg = tl.tile([B, HID], FP32)
                nc.vector.tensor_scalar(out=hg[:], in0=hgt[:],
                                        scalar1=rcnt[:, 0:1], scalar2=None,
                                        op0=ALU.mult)
                hgT = transpose_to(tl, hg[:], B, HID, tag="hg")
                gf_p = ppY.tile([B, HID], FP32, tag="y")
                mm(gf_p[:], hgT[:], par["glin_w"][:], start=True, stop=False)
                mm(gf_p[:], ones_row[:, :B], par["glin_b"][:], start=False,
                   stop=True)

                xcat = tl.tile([B, 3 * HID], FP32)
                nc.vector.tensor_copy(xcat[:, HID : 2 * HID], gf_p[:])

                # embed paths (algo: 3 embeds of 32; sched: 4 embeds of 32)
                def embed_feats(dsrc, ncols, Atab, w1, w1_is_wb, w2w, w2b,
                                outslice, embdim):
                    ep = ppY.tile([B, ncols * embdim], FP32, tag="y")
                    for e in range(ncols):
                        ai = tl.tile([2, B], I32, tag="ai")
                        nc.sync.dma_start(
                            ai[:], bass.AP(dsrc, e, [[0, 2], [ncols, B]]))
                        af = tl.tile([2, B], FP32, tag="af")
                        nc.vector.tensor_copy(af[:], ai[:])
                        ot = tl.tile([2, B], FP32, tag="ot")
                        nc.vector.tensor_tensor(out=ot[:], in0=iop[:2, :],
                                                in1=af[:], op=ALU.is_equal)
                        mm(ep[:, e * embdim : (e + 1) * embdim], ot[:],
                           Atab[:, e * embdim : (e + 1) * embdim])
                    a_sb = tl.tile([B, ncols * embdim], FP32, tag="a_sb")
                    nc.vector.tensor_copy(a_sb[:], ep[:])
                    aT = transpose_to(tl, a_sb[:], B, ncols * embdim, extra=1,
                                      tag="embT")
                    f1 = ppY.tile([B, HID], FP32, tag="y")
                    if w1_is_wb:
                        mm(f1[:], aT[:], w1[:])
                    else:
                        mm(f1[:], aT[: ncols * embdim, :], w1[:], start=True,
                           stop=False)
                        mm(f1[:], ones_row[:, :B], w2b[:], start=False,
                           stop=True)  # unused path
                    f1r = tl.tile([B, HID], FP32, tag="embr")
                    nc.vector.tensor_scalar(out=f1r[:], in0=f1[:], scalar1=0.0,
                                            scalar2=None, op0=ALU.max)
                    f1T = transpose_to(tl, f1r[:], B, HID, tag="embT2")
                    f2 = ppY.tile([B, HID], FP32, tag="y")
                    mm(f2[:], f1T[:], w2w[:], start=True, stop=False)
                    mm(f2[:], ones_row[:, :B], w2b[:], start=False, stop=True)
                    nc.vector.tensor_copy(xcat[:, outslice], f2[:])

                embed_feats(d_algo, 3, par["A_algo"], par["algo1_wb"], True,
                            par["algo2_w"], par["algo2_b"],
                            slice(0, HID), DM)

                # sched1 takes [8,128] input: build s then 128+1 matmul
                ep = ppY.tile([B, HID], FP32, tag="y")
                for e in range(4):
                    ai = tl.tile([2, B], I32, tag="ai")
                    nc.sync.dma_start(ai[:],
                                      bass.AP(d_sched, e, [[0, 2], [4, B]]))
                    af = tl.tile([2, B], FP32, tag="af")
                    nc.vector.tensor_copy(af[:], ai[:])
                    ot = tl.tile([2, B], FP32, tag="ot")
                    nc.vector.tensor_tensor(out=ot[:], in0=iop[:2, :],
                                            in1=af[:], op=ALU.is_equal)
                    mm(ep[:, e * DM : (e + 1) * DM], ot[:],
                       par["A_sched"][:, e * DM : (e + 1) * DM])
                s_sb = tl.tile([B, HID], FP32, tag="a_sb")
                nc.vector.tensor_copy(s_sb[:], ep[:])
                sT = transpose_to(tl, s_sb[:], B, HID, tag="sT")
                sf1 = ppY.tile([B, HID], FP32, tag="y")
                mm(sf1[:], sT[:], par["sched1_w"][:], start=True, stop=False)
                mm(sf1[:], ones_row[:, :B], par["sched1_b"][:], start=False,
                   stop=True)
                sf1r = tl.tile([B, HID], FP32, tag="embr")
                nc.vector.tensor_scalar(out=sf1r[:], in0=sf1[:], scalar1=0.0,
                                        scalar2=None, op0=ALU.max)
                sf1T = transpose_to(tl, sf1r[:], B, HID, tag="embT2")
                sf2 = ppY.tile([B, HID], FP32, tag="y")
                mm(sf2[:], sf1T[:], par["sched2_w"][:], start=True, stop=False)
                mm(sf2[:], ones_row[:, :B], par["sched2_b"][:], start=False,
                   stop=True)
                nc.vector.tensor_copy(xcat[:, 2 * HID : 3 * HID], sf2[:])

                if dbg:
                    nc.sync.dma_start(d_dbg_poolf[:], poolF[:])
                    nc.sync.dma_start(d_dbg_xcat[:], xcat[:])
                # tokens [96, 32] (via DRAM bounce: partition regroup)
                with tc.tile_pool(name="xbounce", bufs=1, space="DRAM") as xb:
                    xcat_d = xb.tile([B, 3 * HID], FP32)
                    nc.sync.dma_start(xcat_d[:], xcat[:])
                    xtok = tl.tile([TOK, DM], FP32, tag="xtok")
                    nc.sync.dma_start(
                        xtok[:],
                        xcat_d[:].rearrange("b (t f) -> (b t) f", f=DM))

                SCL = float(1.0 / np.sqrt(HD))
                for l in range(4):
                    xT = transpose_to(tl, xtok[:], TOK, DM, extra=1,
                                      tag="xT")  # [33, 96]
                    qp = ppY.tile([HD, NH * TOK], FP32, tag="y")
                    kp = ppY.tile([HD, NH * TOK], FP32, tag="y")
                    for h in range(NH):
                        mm(qp[:, h * TOK : (h + 1) * TOK],
                           par[f"tf{l}_q"][:, h * HD : (h + 1) * HD], xT[:])
                        mm(kp[:, h * TOK : (h + 1) * TOK],
                           par[f"tf{l}_k"][:, h * HD : (h + 1) * HD], xT[:])
                    q_sb = tl.tile([HD, NH * TOK], FP32, tag="q_sb")
                    nc.vector.tensor_copy(q_sb[:], qp[:])
                    k_sb = tl.tile([HD, NH * TOK], FP32, tag="k_sb")
                    nc.vector.tensor_copy(k_sb[:], kp[:])
                    vp = ppY.tile([TOK, DM], FP32, tag="y")
                    mm(vp[:], xT[:], par[f"tf{l}_v"][:])
                    v_sb = tl.tile([TOK, DM], FP32, tag="v_sb")
                    nc.vector.tensor_copy(v_sb[:], vp[:])

                    sp = ppX.tile([TOK, NH * TOK], FP32, tag="x")
                    for h in range(NH):
                        mm(sp[:, h * TOK : (h + 1) * TOK],
                           q_sb[:, h * TOK : (h + 1) * TOK],
                           k_sb[:, h * TOK : (h + 1) * TOK])
                    mx = tl.tile([TOK, NH], FP32, tag="mx")
                    nc.vector.tensor_reduce(
                        out=mx[:], in_=sp[:].rearrange("p (h t) -> p h t",
                                                       h=NH),
                        axis=mybir.AxisListType.X, op=ALU.max)
                    nb = tl.tile([TOK, NH], FP32, tag="nb")
                    nc.vector.tensor_scalar(out=nb[:], in0=mx[:],
                                            scalar1=-SCL, scalar2=None,
                                            op0=ALU.mult)
                    P = tl.tile([TOK, NH * TOK], FP32, tag="P")
                    for h in range(NH):
                        nc.scalar.activation(P[:, h * TOK : (h + 1) * TOK],
                                             sp[:, h * TOK : (h + 1) * TOK],
                                             AF.Exp, bias=nb[:, h : h + 1],
                                             scale=SCL)
                    P2 = tl.tile([TOK, NH * TOK], FP32, tag="P2")
                    for h in range(NH):
                        nc.vector.tensor_tensor(
                            out=P2[:, h * TOK : (h + 1) * TOK],
                            in0=P[:, h * TOK : (h + 1) * TOK], in1=gmask[:],
                            op=ALU.mult)
                    rs = tl.tile([TOK, NH], FP32, tag="rs")
                    nc.vector.tensor_reduce(
                        out=rs[:], in_=P2[:].rearrange("p (h t) -> p h t",
                                                       h=NH),
                        axis=mybir.AxisListType.X, op=ALU.add)
                    rr = tl.tile([TOK, NH], FP32, tag="rr")
                    nc.vector.reciprocal(rr[:], rs[:])
                    P3 = tl.tile([TOK, NH * TOK], FP32, tag="P3")
                    for h in range(NH):
                        nc.vector.tensor_scalar(
                            out=P3[:, h * TOK : (h + 1) * TOK],
                            in0=P2[:, h * TOK : (h + 1) * TOK],
                            scalar1=rr[:, h : h + 1], scalar2=None,
                            op0=ALU.mult)
                    op_ = ppY.tile([TOK, DM], FP32, tag="y")
                    for h in range(NH):
                        ptp = ppX.tile([TOK, TOK], FP32, tag="x")
                        nc.tensor.transpose(
                            ptp[:], P3[:, h * TOK : (h + 1) * TOK],
                            idm[:TOK, :TOK])
                        pts = tl.tile([TOK, TOK], FP32, tag="pts")
                        nc.vector.tensor_copy(pts[:], ptp[:])
                        mm(op_[:, h * HD : (h + 1) * HD], pts[:],
                           v_sb[:, h * HD : (h + 1) * HD])
                    o_sb = tl.tile([TOK, DM], FP32, tag="o_sb")
                    nc.vector.tensor_copy(o_sb[:], op_[:])
                    oT = transpose_to(tl, o_sb[:], TOK, DM, extra=1, tag="oT")
                    prj = ppY.tile([TOK, DM], FP32, tag="y")
                    mm(prj[:], oT[:], par[f"tf{l}_o"][:])
                    x1 = tl.tile([TOK, DM], FP32, tag="x1")
                    nc.vector.tensor_tensor(out=x1[:], in0=xtok[:], in1=prj[:],
                                            op=ALU.add)

                    def layernorm(xin, gname, bname, tag):
                        mu = tl.tile([TOK, 1], FP32, tag=f"mu{tag}")
                        nc.vector.tensor_reduce(out=mu[:], in_=xin[:],
                                                axis=mybir.AxisListType.X,
                                                op=ALU.add)
                        nc.vector.tensor_scalar(out=mu[:], in0=mu[:],
                                                scalar1=1.0 / DM,
                                                scalar2=None, op0=ALU.mult)
                        xc = tl.tile([TOK, DM], FP32, tag=f"xc{tag}")
                        nc.vector.tensor_scalar(out=xc[:], in0=xin[:],
                                                scalar1=mu[:, 0:1],
                                                scalar2=None, op0=ALU.subtract)
                        sq = tl.tile([TOK, DM], FP32, tag=f"sq{tag}")
                        nc.vector.tensor_tensor(out=sq[:], in0=xc[:],
                                                in1=xc[:], op=ALU.mult)
                        vr = tl.tile([TOK, 1], FP32, tag=f"vr{tag}")
                        nc.vector.tensor_reduce(out=vr[:], in_=sq[:],
                                                axis=mybir.AxisListType.X,
                                                op=ALU.add)
                        sd = tl.tile([TOK, 1], FP32, tag=f"sd{tag}")
                        nc.scalar.activation(sd[:], vr[:], AF.Sqrt,
                                             bias=eps_col[:TOK, 0:1],
                                             scale=1.0 / DM)
                        rsd = tl.tile([TOK, 1], FP32, tag=f"rsd{tag}")
                        nc.vector.reciprocal(rsd[:], sd[:])
                        xn = tl.tile([TOK, DM], FP32, tag=f"xn{tag}")
                        nc.vector.tensor_scalar(out=xn[:], in0=xc[:],
                                                scalar1=rsd[:, 0:1],
                                                scalar2=None, op0=ALU.mult)
                        xg = tl.tile([TOK, DM], FP32, tag=f"xg{tag}")
                        nc.vector.tensor_tensor(out=xg[:], in0=xn[:],
                                                in1=par[gname][:], op=ALU.mult)
                        xo = tl.tile([TOK, DM], FP32, tag=f"xo{tag}")
                        nc.vector.tensor_tensor(out=xo[:], in0=xg[:],
                                                in1=par[bname][:], op=ALU.add)
                        return xo

                    x2 = layernorm(x1, f"tf{l}_ln1g", f"tf{l}_ln1b", "a")
                    xT2 = transpose_to(tl, x2[:], TOK, DM, extra=1, tag="xT2")
                    f1p = ppX.tile([TOK, FF], FP32, tag="x")
                    mm(f1p[:], xT2[:], par[f"tf{l}_ff1"][:])
                    f1r = tl.tile([TOK, FF], FP32, tag="f1r")
                    nc.vector.tensor_scalar(out=f1r[:], in0=f1p[:],
                                            scalar1=0.0, scalar2=None,
                                            op0=ALU.max)
                    f2p = ppY.tile([TOK, DM], FP32, tag="y")
                    for h2 in range(2):
                        fT = transpose_to(tl, f1r[:, h2 * 128 : (h2 + 1) * 128],
                                          TOK, 128, tag="fT")
                        mm(f2p[:], fT[:], par[f"tf{l}_ff2w"][:, h2, :],
                           start=(h2 == 0), stop=False)
                    mm(f2p[:], ones_row[:, :TOK], par[f"tf{l}_ff2b"][:],
                       start=False, stop=True)
                    x3 = tl.tile([TOK, DM], FP32, tag="x3")
                    nc.vector.tensor_tensor(out=x3[:], in0=x2[:], in1=f2p[:],
                                            op=ALU.add)
                    xtok = tl.tile([TOK, DM], FP32, tag="xtok")
                    xln = layernorm(x3, f"tf{l}_ln2g", f"tf{l}_ln2b", "b")
                    nc.vector.tensor_copy(xtok[:], xln[:])

                # head MLP
                xfin = tl.tile([B, 384], FP32)
                with tc.tile_pool(name="xbounce2", bufs=1, space="DRAM") as xb2:
                    xtok_d = xb2.tile([TOK, DM], FP32)
                    nc.sync.dma_start(xtok_d[:], xtok[:])
                    nc.sync.dma_start(
                        xfin[:],
                        xtok_d[:].rearrange("(b t) f -> b (t f)", b=B))
                m1p = ppY.tile([B, HID], FP32, tag="y")
                for h3 in range(3):
                    xfT = transpose_to(tl, xfin[:, h3 * 128 : (h3 + 1) * 128],
                                       B, 128, tag="xfT")
                    mm(m1p[:], xfT[:], par["mlp1_w"][:, h3, :],
                       start=(h3 == 0), stop=False)
                mm(m1p[:], ones_row[:, :B], par["mlp1_b"][:], start=False,
                   stop=True)
                m1r = tl.tile([B, HID], FP32)
                nc.vector.tensor_scalar(out=m1r[:], in0=m1p[:], scalar1=0.0,
                                        scalar2=None, op0=ALU.max)
                m1T = transpose_to(tl, m1r[:], B, HID, tag="m1T")
                m2p = ppY.tile([B, 64], FP32, tag="y")
                mm(m2p[:], m1T[:], par["mlp2_w"][:], start=True, stop=False)
                mm(m2p[:], ones_row[:, :B], par["mlp2_b"][:], start=False,
                   stop=True)
                m2r = tl.tile([B, 64], FP32)
                nc.vector.tensor_scalar(out=m2r[:], in0=m2p[:], scalar1=0.0,
                                        scalar2=None, op0=ALU.max)
                m2T = transpose_to(tl, m2r[:], B, 64, extra=1, tag="m2T")
                m3p = ppY.tile([B, 1], FP32, tag="y")
                mm(m3p[:], m2T[:], par["mlp3_wb"][:])
                res = tl.tile([B, 1], FP32)
                nc.vector.tensor_copy(res[:], m3p[:])
                nc.sync.dma_start(d_out[:], res[:])

    nc.compile()

    input_specs = {}
    for name, h in ins.items():
        input_specs[name] = name
    return nc


# ----------------------------------------------------------------------------
# Runner
# ----------------------------------------------------------------------------

_CACHE = {}
_PREP_CACHE = {}


def make_in_maps(cfg, rep, per_core, hp, algo_ops, schedule):
    algo = np.ascontiguousarray(np.asarray(algo_ops, dtype=np.int32))
    sched = np.ascontiguousarray(np.asarray(schedule, dtype=np.int32))
    in_maps = []
    for c in range(NCORES):
        m = dict(rsD=rep["rsD"], reD=rep["reD"], rsS=rep["rsS"],
                 reS=rep["reS"], gid=rep["gid"], algo=algo, sched=sched)
        m.update(per_core[c])
        m.update(hp)
        in_maps.append(m)
    return in_maps


def kernel(algo_ops, schedule, edge_src, edge_dst, node2graph, params,
           n_true=None):
    from concourse.bass_utils import run_bass_kernel_spmd

    import hashlib

    edge_src = np.ascontiguousarray(np.asarray(edge_src, dtype=np.int32))
    edge_dst = np.ascontiguousarray(np.asarray(edge_dst, dtype=np.int32))
    node2graph = np.ascontiguousarray(np.asarray(node2graph, dtype=np.int32))
    if n_true is None:
        n_true = int(node2graph.shape[0])
    E = int(edge_src.shape[0])
    h = hashlib.blake2b(digest_size=16)
    h.update(edge_src.tobytes())
    h.update(edge_dst.tobytes())
    h.update(node2graph.tobytes())
    pkey = (n_true, E, h.hexdigest())
    if pkey in _PREP_CACHE:
        cfg, rep, per_core = _PREP_CACHE[pkey]
    else:
        cfg, rep, per_core = host_prep(edge_src, edge_dst, node2graph,
                                       n_true, E)
        _PREP_CACHE.clear()
        _PREP_CACHE[pkey] = (cfg, rep, per_core)
    hp = host_params(params)

    edge_fp16 = os.environ.get("GNN_EDGE_FP16", "0") == "1"
    key = (cfg.NT, cfg.E, tuple(cfg.KW), edge_fp16)
    if key not in _CACHE:
        _CACHE[key] = build_program(cfg, edge_fp16=edge_fp16)
    nc = _CACHE[key]

    in_maps = make_in_maps(cfg, rep, per_core, hp, algo_ops, schedule)
    trace = os.environ.get("GNN_TRACE", "0") == "1"
    if trace:
        try:
            import antenv.axon_hooks  # noqa: F401  (NTFF hook availability)
        except ImportError:
            trace = False
    res = run_bass_kernel_spmd(nc, in_maps, core_ids=list(range(NCORES)),
                               trace=trace)
    out = res.results[0]["out"]
    if trace and res.exec_time_ns is not None:
        print(f"HW exec time: {res.exec_time_ns} ns")
    return np.asarray(out, dtype=np.float32)


# revision 33
# speedup vs baseline: 1.2443x; 1.2443x over previous
"""Trainium2 Bass kernel for nn_AutoGraphModel (GNN message passing).

Strategy (8 NeuronCores, SPMD):
  - Nodes are range-sharded across cores in 128-aligned blocks; edges are
    sharded by destination node and sorted by dst on the host (layout only).
  - Per core, dst-range edges are processed in 128-edge tiles grouped into
    128-node "windows"; segment-sum (scatter-add) is done race-free on the
    TensorEngine via one-hot selection matmuls accumulating in PSUM.
  - gconv1 gathers 2-float node features via indirect DMA (8B rows);
    gconv2 gathers 128-float rows of the (replicated) h1 table.
  - h1 is computed sharded and replicated with an AllGather; graph pooling
    partials are combined with an AllReduce (the "psum" boundary).
  - The tiny transformer/MLP head runs replicated on every core.

Host-side work is strictly integer layout/sharding: sorting edge lists,
CSR rowptr construction, padding, and concatenating parameter arrays.
All floating-point math happens on device.
"""

import os
import numpy as np

import concourse.bacc as bacc
import concourse.bass as bass
import concourse.mybir as mybir
import concourse.tile as tile
from concourse.masks import make_identity

FP32 = mybir.dt.float32
FP16 = mybir.dt.float16
I32 = mybir.dt.int32
AF = mybir.ActivationFunctionType
ALU = mybir.AluOpType

NCORES = 8
B = 8
DM, NH, HD, HID, FF = 32, 4, 8, 128, 256
TOK = B * 12  # 96 tokens


class Cfg:
    def __init__(self, n_true, n_edges, KW):
        self.NT = n_true
        self.E = n_edges
        per = -(-n_true // NCORES)            # ceil
        self.RANGE = -(-per // 128) * 128     # 128-aligned nodes per core
        self.W = self.RANGE // 128            # windows per core
        self.NPAD = NCORES * self.RANGE
        self.XN = self.NPAD // 128
        self.KW = list(KW)                    # tiles per window (max over cores)
        self.TSTART = np.concatenate([[0], np.cumsum(self.KW)]).astype(int)
        self.T = int(self.TSTART[-1])         # edge tiles per core


# ----------------------------------------------------------------------------
# Host-side integer preprocessing (layout / sharding only)
# ----------------------------------------------------------------------------

def host_prep(edge_src, edge_dst, node2graph, n_true, n_edges):
    es = np.asarray(edge_src, dtype=np.int64)
    ed = np.asarray(edge_dst, dtype=np.int64)
    n2g = np.asarray(node2graph, dtype=np.int32)
    N = n_true

    order = np.argsort(ed, kind="stable")
    ed_s = ed[order]
    es_s = es[order].astype(np.int32)

    cnt_dst = np.bincount(ed, minlength=N)
    cnt_src = np.bincount(es, minlength=N)
    rp_dst = np.concatenate([[0], np.cumsum(cnt_dst)]).astype(np.int32)
    rp_src = np.concatenate([[0], np.cumsum(cnt_src)]).astype(np.int32)

    # First pass: per-window tile counts (max over cores)
    cfg0 = Cfg(n_true, n_edges, [1])
    RANGE, W, NPAD, XN = cfg0.RANGE, cfg0.W, cfg0.NPAD, cfg0.XN

    KW = np.ones(W, dtype=np.int64)
    core_edges = []
    for c in range(NCORES):
        base = c * RANGE
        lo = np.searchsorted(ed_s, base, side="left")
        hi = np.searchsorted(ed_s, base + RANGE, side="left")
        dl_all = (ed_s[lo:hi] - base).astype(np.int32)
        src_all = es_s[lo:hi]
        win = dl_all >> 7
        wcnt = np.bincount(win, minlength=W)
        KW = np.maximum(KW, -(-wcnt // 128))
        core_edges.append((dl_all, src_all, win, wcnt))

    cfg = Cfg(n_true, n_edges, KW)
    T = cfg.T
    TSTART = cfg.TSTART

    def layout_128(arr_1d, npad, fill):
        out = np.full(npad, fill, dtype=np.int32)
        out[: arr_1d.shape[0]] = arr_1d
        return np.ascontiguousarray(out.reshape(-1, 128).T)  # [128, npad/128]

    rsD = layout_128(rp_dst[:N], NPAD, rp_dst[N])
    reD = layout_128(rp_dst[1 : N + 1], NPAD, rp_dst[N])
    rsS = layout_128(rp_src[:N], NPAD, rp_src[N])
    reS = layout_128(rp_src[1 : N + 1], NPAD, rp_src[N])

    per_core = []
    for c in range(NCORES):
        dl_all, src_all, win, wcnt = core_edges[c]
        ne = dl_all.shape[0]
        wstart = np.concatenate([[0], np.cumsum(wcnt)])
        j_in_w = np.arange(ne, dtype=np.int64) - wstart[win]
        pos = TSTART[win] * 128 + j_in_w
        esrc_flat = np.zeros(T * 128, dtype=np.int32)
        edl_flat = np.full(T * 128, -1, dtype=np.int32)
        esrc_flat[pos] = src_all
        edl_flat[pos] = dl_all & 127
        esrc_sb = np.ascontiguousarray(esrc_flat.reshape(T, 128).T)  # [128,T]
        edl_sb = np.ascontiguousarray(edl_flat.reshape(T, 128).T)

        base = c * RANGE

        def own_layout(arr_np1, fill):
            seg = np.full(RANGE, fill, dtype=np.int32)
            m = max(0, min(RANGE, N - base))
            seg[:m] = arr_np1[base : base + m]
            return np.ascontiguousarray(seg.reshape(W, 128).T)  # [128, W]

        rsD_o = own_layout(rp_dst[:N], rp_dst[N])
        reD_o = own_layout(rp_dst[1 : N + 1], rp_dst[N])
        rsS_o = own_layout(rp_src[:N], rp_src[N])
        reS_o = own_layout(rp_src[1 : N + 1], rp_src[N])
        n2g_o = own_layout(n2g, -1)
        per_core.append(
            dict(esrc=esrc_sb, edl=edl_sb, rsD_o=rsD_o, reD_o=reD_o,
                 rsS_o=rsS_o, reS_o=reS_o, n2g_o=n2g_o)
        )

    gid = np.ascontiguousarray((np.arange(TOK, dtype=np.int32) // 12)[None, :])
    rep = dict(rsD=rsD, reD=reD, rsS=rsS, reS=reS, gid=gid)
    return cfg, rep, per_core


def host_params(params):
    """Concatenate parameter arrays into device layouts (pure memcpy)."""
    def A(x):
        return np.ascontiguousarray(np.asarray(x, dtype=np.float32))

    def wb(p):  # [K+1, M] = [W; b]
        return np.concatenate([A(p["W"]), A(p["b"])[None, :]], axis=0)

    out = {}
    out["gc1_w"] = A(params["gc1"]["W"])                    # [2, 128]
    out["gc1_b"] = A(params["gc1"]["b"])[None, :]           # [1, 128]
    out["gc2_w"] = A(params["gc2"]["W"])                    # [128, 128]
    out["gc2_b"] = A(params["gc2"]["b"])[None, :]           # [1, 128]
    out["glin_w"] = A(params["graph_lin"]["W"])             # [128, 128]
    out["glin_b"] = A(params["graph_lin"]["b"])[None, :]
    out["algo1_wb"] = wb(params["algo1"])                   # [97, 128]
    out["algo2_w"] = A(params["algo2"]["W"])
    out["algo2_b"] = A(params["algo2"]["b"])[None, :]
    out["sched1_w"] = A(params["sched1"]["W"])
    out["sched1_b"] = A(params["sched1"]["b"])[None, :]
    out["sched2_w"] = A(params["sched2"]["W"])
    out["sched2_b"] = A(params["sched2"]["b"])[None, :]
    out["A_algo"] = np.concatenate(
        [A(params["msg_create"])[:2], A(params["msg_reduce"])[:2],
         A(params["compute_mode"])[:2]], axis=1)           # [2, 96]
    out["A_sched"] = np.concatenate(
        [A(params["direction"])[:2], A(params["parallel"])[:2],
         A(params["frontier"])[:2], A(params["ssg"])[:2]], axis=1)  # [2,128]
    for l, lp in enumerate(params["tf"]):
        out[f"tf{l}_q"] = wb(lp["q"])       # [33, 32]
        out[f"tf{l}_k"] = wb(lp["k"])
        out[f"tf{l}_v"] = wb(lp["v"])
        out[f"tf{l}_o"] = wb(lp["o"])
        out[f"tf{l}_ff1"] = wb(lp["ff1"])   # [33, 256]
        out[f"tf{l}_ff2w"] = np.ascontiguousarray(
            A(lp["ff2"]["W"]).reshape(2, 128, DM).transpose(1, 0, 2))
        out[f"tf{l}_ff2b"] = A(lp["ff2"]["b"])[None, :]     # [1, 32]
        out[f"tf{l}_ln1g"] = np.broadcast_to(A(lp["ln1_g"])[None, :], (TOK, DM)).copy()
        out[f"tf{l}_ln1b"] = np.broadcast_to(A(lp["ln1_b"])[None, :], (TOK, DM)).copy()
        out[f"tf{l}_ln2g"] = np.broadcast_to(A(lp["ln2_g"])[None, :], (TOK, DM)).copy()
        out[f"tf{l}_ln2b"] = np.broadcast_to(A(lp["ln2_b"])[None, :], (TOK, DM)).copy()
    out["mlp1_w"] = np.ascontiguousarray(
        A(params["mlp1"]["W"]).reshape(3, 128, HID).transpose(1, 0, 2))
    out["mlp1_b"] = A(params["mlp1"]["b"])[None, :]
    out["mlp2_w"] = A(params["mlp2"]["W"])   # [128, 64]
    out["mlp2_b"] = A(params["mlp2"]["b"])[None, :]
    out["mlp3_wb"] = wb(params["mlp3"])      # [65, 1]
    return out


# ----------------------------------------------------------------------------
# Device program
# ----------------------------------------------------------------------------

PARAM_SHAPES = None  # filled by build_program


def build_program(cfg, edge_fp16=True):
    """Builds the SPMD Bass program. Returns (nc, input_specs) where
    input_specs maps tensor-name -> (shape, np dtype)."""
    W, T, XN, NPAD, RANGE = cfg.W, cfg.T, cfg.XN, cfg.NPAD, cfg.RANGE
    KW, TSTART = cfg.KW, cfg.TSTART
    EDT = FP16 if edge_fp16 else FP32
    EDT_np = np.float16 if edge_fp16 else np.float32

    nc = bacc.Bacc("TRN2", target_bir_lowering=False, debug=False,
                   num_devices=NCORES)

    ins = {}

    def di(name, shape, dt):
        h = nc.dram_tensor(name, list(shape), dt, kind="ExternalInput")
        ins[name] = h
        return h

    # int inputs
    d_rsD = di("rsD", (128, XN), I32)
    d_reD = di("reD", (128, XN), I32)
    d_rsS = di("rsS", (128, XN), I32)
    d_reS = di("reS", (128, XN), I32)
    d_gid = di("gid", (1, TOK), I32)
    d_algo = di("algo", (B, 3), I32)
    d_sched = di("sched", (B, 4), I32)
    d_esrc = di("esrc", (128, T), I32)
    d_edl = di("edl", (128, T), I32)
    d_rsD_o = di("rsD_o", (128, W), I32)
    d_reD_o = di("reD_o", (128, W), I32)
    d_rsS_o = di("rsS_o", (128, W), I32)
    d_reS_o = di("reS_o", (128, W), I32)
    d_n2g_o = di("n2g_o", (128, W), I32)

    # parameter inputs
    pshapes = {
        "gc1_w": (2, HID), "gc1_b": (1, HID),
        "gc2_w": (HID, HID), "gc2_b": (1, HID),
        "glin_w": (HID, HID), "glin_b": (1, HID),
        "algo1_wb": (97, HID), "algo2_w": (HID, HID), "algo2_b": (1, HID),
        "sched1_w": (HID, HID), "sched1_b": (1, HID),
        "sched2_w": (HID, HID), "sched2_b": (1, HID),
        "A_algo": (2, 96), "A_sched": (2, HID),
        "mlp1_w": (HID, 3, HID), "mlp1_b": (1, HID),
        "mlp2_w": (HID, 64), "mlp2_b": (1, 64), "mlp3_wb": (65, 1),
    }
    for l in range(4):
        pshapes[f"tf{l}_q"] = (33, DM)
        pshapes[f"tf{l}_k"] = (33, DM)
        pshapes[f"tf{l}_v"] = (33, DM)
        pshapes[f"tf{l}_o"] = (33, DM)
        pshapes[f"tf{l}_ff1"] = (33, FF)
        pshapes[f"tf{l}_ff2w"] = (HID, 2, DM)
        pshapes[f"tf{l}_ff2b"] = (1, DM)
        for nm in ("ln1g", "ln1b", "ln2g", "ln2b"):
            pshapes[f"tf{l}_{nm}"] = (TOK, DM)
    dpar = {k: di(k, v, FP32) for k, v in pshapes.items()}

    # internal DRAM
    d_vtab = nc.dram_tensor("vtab", [NPAD, 2], FP32)
    d_h1own = nc.dram_tensor("h1own", [RANGE, HID], EDT)
    d_h1full = nc.dram_tensor("h1full", [NPAD, HID], EDT, addr_space="Shared")
    d_poolp = nc.dram_tensor("poolp", [B, HID + 1], FP32)
    d_poolf = nc.dram_tensor("poolf", [B, HID + 1], FP32, addr_space="Shared")
    d_out = nc.dram_tensor("out", [B, 1], FP32, kind="ExternalOutput")
    dbg = os.environ.get("GNN_DBG", "0") == "1"
    if dbg:
        d_dbg_h1 = nc.dram_tensor("dbg_h1", [RANGE, HID], FP32,
                                  kind="ExternalOutput")
        d_dbg_pool = nc.dram_tensor("dbg_pool", [B, HID + 1], FP32,
                                    kind="ExternalOutput")
        d_dbg_poolf = nc.dram_tensor("dbg_poolf", [B, HID + 1], FP32,
                                     kind="ExternalOutput")
        d_dbg_xcat = nc.dram_tensor("dbg_xcat", [B, 3 * HID], FP32,
                                    kind="ExternalOutput")
        d_dbg_vtab = nc.dram_tensor("dbg_vtab", [NPAD, 2], FP32,
                                    kind="ExternalOutput")

    rg = [list(range(NCORES))]

    with tile.TileContext(nc) as tc:
        with tc.tile_pool(name="const", bufs=1) as cpool:
            # --- constants / params in SBUF ---
            idm = cpool.tile([128, 128], FP32)
            make_identity(nc, idm[:])
            ones_row = cpool.tile([1, 128], FP32)
            nc.vector.memset(ones_row[:], 1.0)
            eps_col = cpool.tile([128, 1], FP32)
            nc.vector.memset(eps_col[:], 1e-5)
            iota_i = cpool.tile([128, 128], I32)
            nc.gpsimd.iota(iota_i[:], pattern=[[1, 128]], base=0,
                           channel_multiplier=0)
            iota_e = cpool.tile([128, 128], EDT)
            nc.vector.tensor_copy(iota_e[:], iota_i[:])
            iota8 = cpool.tile([128, 8], FP32)
            nc.vector.tensor_copy(iota8[:], iota_i[:, :8])
            # iota over partitions (value = partition index), for one-hots
            iop_i = cpool.tile([128, B], I32)
            nc.gpsimd.iota(iop_i[:], pattern=[[0, B]], base=0,
                           channel_multiplier=1)
            iop = cpool.tile([128, B], FP32)
            nc.vector.tensor_copy(iop[:], iop_i[:])

            par = {}
            for k, sh in pshapes.items():
                t_ = cpool.tile(list(sh), FP32, tag=f"p_{k}")
                nc.sync.dma_start(t_[:], dpar[k][:])
                par[k] = t_

            # token-graph mask [96, 96]
            gcol_i = cpool.tile([TOK, 1], I32)
            nc.sync.dma_start(gcol_i[:],
                              bass.AP(d_gid, 0, [[1, TOK], [1, 1]]))
            grow_i = cpool.tile([TOK, TOK], I32)
            nc.sync.dma_start(grow_i[:], bass.AP(d_gid, 0, [[0, TOK], [1, TOK]]))
            gmask = cpool.tile([TOK, TOK], FP32)
            nc.vector.tensor_tensor(out=gmask[:], in0=grow_i[:],
                                    in1=gcol_i[:].to_broadcast([TOK, TOK]),
                                    op=ALU.is_equal)

            # --- degrees, norms, v-table ---
            with tc.tile_pool(name="deg", bufs=1) as dp:
                _dgc = [0]
                def deg_f32(pool, rs, re, n):
                    _dgc[0] += 1
                    ti = pool.tile([128, n], I32, tag=f"deg_i{_dgc[0]}")
                    nc.vector.tensor_tensor(out=ti[:], in0=re[:], in1=rs[:],
                                            op=ALU.subtract)
                    tf = pool.tile([128, n], FP32, tag=f"deg_f{_dgc[0]}")
                    nc.vector.tensor_scalar(out=tf[:], in0=ti[:], scalar1=1.0,
                                            scalar2=None, op0=ALU.add)
                    return tf

                rsD_s = dp.tile([128, XN], I32, tag="rp1")
                reD_s = dp.tile([128, XN], I32, tag="rp2")
                rsS_s = dp.tile([128, XN], I32, tag="rp3")
                reS_s = dp.tile([128, XN], I32, tag="rp4")
                nc.sync.dma_start(rsD_s[:], d_rsD[:])
                nc.sync.dma_start(reD_s[:], d_reD[:])
                nc.sync.dma_start(rsS_s[:], d_rsS[:])
                nc.sync.dma_start(reS_s[:], d_reS[:])
                din = deg_f32(dp, rsD_s, reD_s, XN)
                dout = deg_f32(dp, rsS_s, reS_s, XN)
                tmp = dp.tile([128, XN], FP32)
                nc.scalar.activation(tmp[:], dout[:], AF.Sqrt)
                dno = dp.tile([128, XN], FP32)
                nc.vector.reciprocal(dno[:], tmp[:])
                vbuf = dp.tile([128, XN, 2], FP32)
                nc.vector.tensor_tensor(out=vbuf[:, :, 0], in0=dno[:],
                                        in1=din[:], op=ALU.mult)
                nc.vector.tensor_tensor(out=vbuf[:, :, 1], in0=dno[:],
                                        in1=dout[:], op=ALU.mult)
                nc.sync.dma_start(
                    d_vtab[:].rearrange("(x p) f -> p x f", p=128), vbuf[:])

            # --- own-range node data (persist through B and C) ---
            rsDo_s = cpool.tile([128, W], I32, tag="rpo1")
            reDo_s = cpool.tile([128, W], I32, tag="rpo2")
            rsSo_s = cpool.tile([128, W], I32, tag="rpo3")
            reSo_s = cpool.tile([128, W], I32, tag="rpo4")
            nc.sync.dma_start(rsDo_s[:], d_rsD_o[:])
            nc.sync.dma_start(reDo_s[:], d_reD_o[:])
            nc.sync.dma_start(rsSo_s[:], d_rsS_o[:])
            nc.sync.dma_start(reSo_s[:], d_reS_o[:])

            dinO_i = cpool.tile([128, W], I32)
            nc.vector.tensor_tensor(out=dinO_i[:], in0=reDo_s[:],
                                    in1=rsDo_s[:], op=ALU.subtract)
            dinO = cpool.tile([128, W], FP32)
            nc.vector.tensor_scalar(out=dinO[:], in0=dinO_i[:], scalar1=1.0,
                                    scalar2=None, op0=ALU.add)
            doutO_i = cpool.tile([128, W], I32)
            nc.vector.tensor_tensor(out=doutO_i[:], in0=reSo_s[:],
                                    in1=rsSo_s[:], op=ALU.subtract)
            doutO = cpool.tile([128, W], FP32)
            nc.vector.tensor_scalar(out=doutO[:], in0=doutO_i[:], scalar1=1.0,
                                    scalar2=None, op0=ALU.add)
            tmpO = cpool.tile([128, W], FP32, tag="tmpO")
            nc.scalar.activation(tmpO[:], doutO[:], AF.Sqrt)
            dnoO = cpool.tile([128, W], FP32)
            nc.vector.reciprocal(dnoO[:], tmpO[:])
            tmpO2 = cpool.tile([128, W], FP32, tag="tmpO")
            nc.scalar.activation(tmpO2[:], dinO[:], AF.Sqrt)
            dniO = cpool.tile([128, W], FP32)
            nc.vector.reciprocal(dniO[:], tmpO2[:])
            vO = cpool.tile([128, W, 2], FP32)
            nc.vector.tensor_tensor(out=vO[:, :, 0], in0=dnoO[:], in1=dinO[:],
                                    op=ALU.mult)
            nc.vector.tensor_tensor(out=vO[:, :, 1], in0=dnoO[:], in1=doutO[:],
                                    op=ALU.mult)
            n2gF = cpool.tile([128, W], FP32)

            # edge metadata
            esrc_sb = cpool.tile([128, T], I32)
            nc.sync.dma_start(esrc_sb[:], d_esrc[:])
            edl_sb = cpool.tile([128, T], I32)
            nc.sync.dma_start(edl_sb[:], d_edl[:])
            edl_e = cpool.tile([128, T], FP32)
            nc.vector.tensor_copy(edl_e[:], edl_sb[:])

            n2g_i = cpool.tile([128, W], I32)
            nc.sync.dma_start(n2g_i[:], d_n2g_o[:])
            nc.vector.tensor_copy(n2gF[:], n2g_i[:])

            h1own_sb = cpool.tile([128, W, HID], FP32)  # 50KB/part

            # ---------------- phase B: gconv1 ----------------
            with tc.tile_pool(name="pbv", bufs=16) as pbv, \
                 tc.tile_pool(name="pbs", bufs=4) as pbs, \
                 tc.tile_pool(name="ppB", bufs=2, space="PSUM") as ppB, \
                 tc.tile_pool(name="ppT", bufs=2, space="PSUM") as ppT, \
                 tc.tile_pool(name="ppZ", bufs=2, space="PSUM") as ppZ:
                for w in range(W):
                    ps = ppB.tile([128, 2], FP32)
                    kw = KW[w]
                    for k in range(kw):
                        t = int(TSTART[w]) + k
                        gv = pbv.tile([128, 2], FP32, tag="gv")
                        nc.gpsimd.indirect_dma_start(
                            out=gv[:], out_offset=None, in_=d_vtab[:],
                            in_offset=bass.IndirectOffsetOnAxis(
                                ap=esrc_sb[:, t : t + 1], axis=0))
                        S = pbs.tile([128, 128], FP32, tag="SB")
                        nc.vector.tensor_scalar(
                            out=S[:], in0=iota_e[:],
                            scalar1=edl_e[:, t : t + 1], scalar2=None,
                            op0=ALU.is_equal)
                        nc.tensor.matmul(
                            ps[:], lhsT=S[:], rhs=gv[:],
                            start=(k == 0), stop=(k == kw - 1))
                    # drain: (+ self loop) * dn_in
                    t1 = pbs.tile([128, 2], FP32, tag="dr1")
                    nc.vector.tensor_tensor(out=t1[:], in0=ps[:],
                                            in1=vO[:, w, :], op=ALU.add)
                    t2 = pbs.tile([128, 2], FP32, tag="dr2")
                    nc.vector.tensor_scalar(out=t2[:], in0=t1[:],
                                            scalar1=dniO[:, w : w + 1],
                                            scalar2=None, op0=ALU.mult)
                    pt = ppT.tile([2, 128], FP32)
                    nc.tensor.transpose(pt[:], t2[:], idm[:])
                    lz = pbs.tile([2, 128], FP32, tag="lz")
                    nc.vector.tensor_copy(lz[:], pt[:])
                    zp = ppZ.tile([128, HID], FP32)
                    nc.tensor.matmul(zp[:], lhsT=lz[:], rhs=par["gc1_w"][:],
                                     start=True, stop=False)
                    nc.tensor.matmul(zp[:], lhsT=ones_row[:],
                                     rhs=par["gc1_b"][:],
                                     start=False, stop=True)
                    rt = pbs.tile([128, HID], FP32, tag="rt")
                    nc.vector.tensor_scalar(out=rt[:], in0=zp[:], scalar1=0.0,
                                            scalar2=None, op0=ALU.max)
                    mt = pbs.tile([128, HID], FP32, tag="mt")
                    nc.vector.tensor_scalar(out=mt[:], in0=zp[:], scalar1=0.0,
                                            scalar2=None, op0=ALU.min)
                    et = pbs.tile([128, HID], FP32, tag="et")
                    nc.scalar.activation(et[:], mt[:], AF.Exp)
                    st = pbs.tile([128, HID], FP32, tag="st")
                    nc.vector.tensor_tensor(out=st[:], in0=rt[:], in1=et[:],
                                            op=ALU.add)
                    nc.vector.tensor_scalar(out=h1own_sb[:, w, :], in0=st[:],
                                            scalar1=-1.0,
                                            scalar2=dnoO[:, w : w + 1],
                                            op0=ALU.add, op1=ALU.mult)

            # h1own -> DRAM (cast to EDT) and AllGather
            if EDT == FP32:
                # no cast needed: HWDGE keeps this off the Pool engine,
                # which is saturated by the gather descriptor stream
                nc.sync.dma_start(
                    out=d_h1own[:].rearrange("(w p) f -> p w f", p=128),
                    in_=h1own_sb[:])
            else:
                nc.gpsimd.dma_start(
                    out=d_h1own[:].rearrange("(w p) f -> p w f", p=128),
                    in_=h1own_sb[:])
            nc.gpsimd.collective_compute(
                "AllGather", ALU.bypass, ins=[d_h1own[:]], outs=[d_h1full[:]],
                replica_groups=rg)

            # ---------------- phase C: gconv2 + pooling ----------------
            with tc.tile_pool(name="pcg", bufs=16) as pcg, \
                 tc.tile_pool(name="pcs", bufs=4) as pcs, \
                 tc.tile_pool(name="ppC", bufs=2, space="PSUM") as ppC, \
                 tc.tile_pool(name="ppT2", bufs=2, space="PSUM") as ppT2, \
                 tc.tile_pool(name="ppZ2", bufs=2, space="PSUM") as ppZ2, \
                 tc.tile_pool(name="ppPool", bufs=1, space="PSUM") as ppPool:
                poolp = ppPool.tile([B, HID + 1], FP32)
                for w in range(W):
                    ps = ppC.tile([128, HID], FP32)
                    kw = KW[w]
                    for k in range(kw):
                        t = int(TSTART[w]) + k
                        gh = pcg.tile([128, HID], FP32, tag="gh")
                        nc.gpsimd.indirect_dma_start(
                            out=gh[:], out_offset=None, in_=d_h1full[:],
                            in_offset=bass.IndirectOffsetOnAxis(
                                ap=esrc_sb[:, t : t + 1], axis=0))
                        S = pcs.tile([128, 128], FP32, tag="SC")
                        nc.vector.tensor_scalar(
                            out=S[:], in0=iota_e[:],
                            scalar1=edl_e[:, t : t + 1], scalar2=None,
                            op0=ALU.is_equal)
                        nc.tensor.matmul(
                            ps[:], lhsT=S[:], rhs=gh[:],
                            start=(k == 0), stop=(k == kw - 1))
                    # drain
                    a1 = pcs.tile([128, HID], FP32, tag="a1")
                    nc.vector.tensor_tensor(out=a1[:], in0=ps[:],
                                            in1=h1own_sb[:, w, :], op=ALU.add)
                    a2 = pcs.tile([128, HID], FP32, tag="a2")
                    nc.vector.tensor_scalar(out=a2[:], in0=a1[:],
                                            scalar1=dniO[:, w : w + 1],
                                            scalar2=None, op0=ALU.mult)
                    pt = ppT2.tile([128, HID], FP32)
                    nc.tensor.transpose(pt[:], a2[:], idm[:])
                    lz = pcs.tile([128, HID], FP32, tag="lz2")
                    nc.vector.tensor_copy(lz[:], pt[:])
                    zp = ppZ2.tile([128, HID], FP32)
                    nc.tensor.matmul(zp[:], lhsT=lz[:], rhs=par["gc2_w"][:],
                                     start=True, stop=False)
                    nc.tensor.matmul(zp[:], lhsT=ones_row[:],
                                     rhs=par["gc2_b"][:],
                                     start=False, stop=True)
                    rt = pcs.tile([128, HID], FP32, tag="rt2")
                    nc.vector.tensor_scalar(out=rt[:], in0=zp[:], scalar1=0.0,
                                            scalar2=None, op0=ALU.max)
                    mt = pcs.tile([128, HID], FP32, tag="mt2")
                    nc.vector.tensor_scalar(out=mt[:], in0=zp[:], scalar1=0.0,
                                            scalar2=None, op0=ALU.min)
                    et = pcs.tile([128, HID], FP32, tag="et2")
                    nc.scalar.activation(et[:], mt[:], AF.Exp)
                    h2t = pcs.tile([128, HID + 1], FP32, tag="h2t")
                    nc.vector.tensor_tensor(out=h2t[:, :HID], in0=rt[:],
                                            in1=et[:], op=ALU.add)
                    nc.vector.memset(h2t[:, HID : HID + 1], 1.0)
                    G = pcs.tile([128, B], FP32, tag="G")
                    nc.vector.tensor_scalar(out=G[:], in0=iota8[:],
                                            scalar1=n2gF[:, w : w + 1],
                                            scalar2=None, op0=ALU.is_equal)
                    nc.tensor.matmul(poolp[:], lhsT=G[:], rhs=h2t[:],
                                     start=(w == 0), stop=(w == W - 1),
                                     skip_group_check=True)

                pool_sb = pcs.tile([B, HID + 1], FP32, tag="poolsb")
                nc.vector.tensor_copy(pool_sb[:], poolp[:])
                nc.sync.dma_start(d_poolp[:], pool_sb[:])
                if dbg:
                    nc.sync.dma_start(d_dbg_pool[:], pool_sb[:])

            nc.gpsimd.collective_compute(
                "AllReduce", ALU.add, ins=[d_poolp[:]], outs=[d_poolf[:]],
                replica_groups=rg)

            # ---------------- tail (replicated on every core) --------------
            with tc.tile_pool(name="tl", bufs=2) as tl, \
                 tc.tile_pool(name="ppX", bufs=3, space="PSUM") as ppX, \
                 tc.tile_pool(name="ppY", bufs=3, space="PSUM") as ppY:

                def mm(out_ap, lhs_ap, rhs_ap, start=True, stop=True):
                    nc.tensor.matmul(out_ap, lhsT=lhs_ap, rhs=rhs_ap,
                                     start=start, stop=stop)

                def transpose_to(pool, in_ap, pdim, fdim, extra=0, tag=None):
                    """Return SBUF tile [fdim(+extra), pdim] holding in_ap^T
                    (+ optional ones row at index fdim)."""
                    pt = ppX.tile([fdim, pdim], FP32, tag="x")
                    nc.tensor.transpose(pt[:], in_ap, idm[:pdim, :pdim])
                    st = pool.tile([fdim + extra, pdim], FP32, tag=f"Ts{tag}")
                    nc.vector.tensor_copy(st[:fdim, :], pt[:])
                    if extra:
                        nc.vector.memset(st[fdim : fdim + 1, :], 1.0)
                    return st

                # pooled graph feature
                poolF = tl.tile([B, HID + 1], FP32)
                nc.sync.dma_start(poolF[:], d_poolf[:])
                cntm = tl.tile([B, 1], FP32)
                nc.vector.tensor_scalar(out=cntm[:],
                                        in0=poolF[:, HID : HID + 1],
                                        scalar1=1.0, scalar2=None, op0=ALU.max)
                rcnt = tl.tile([B, 1], FP32)
                nc.vector.reciprocal(rcnt[:], cntm[:])
                hgt = tl.tile([B, HID], FP32)
                nc.vector.tensor_scalar(out=hgt[:], in0=poolF[:, :HID],
                                        scalar1=poolF[:, HID : HID + 1],
                                        scalar2=None, op0=ALU.subtract)
                hg = tl.tile([B, HID], FP32)
                nc.vector.tensor_scalar(out=hg[:], in0=hgt[:],
                                        scalar1=rcnt[:, 0:1], scalar2=None,
                                        op0=ALU.mult)
                hgT = transpose_to(tl, hg[:], B, HID, tag="hg")
                gf_p = ppY.tile([B, HID], FP32, tag="y")
                mm(gf_p[:], hgT[:], par["glin_w"][:], start=True, stop=False)
                mm(gf_p[:], ones_row[:, :B], par["glin_b"][:], start=False,
                   stop=True)

                xcat = tl.tile([B, 3 * HID], FP32)
                nc.vector.tensor_copy(xcat[:, HID : 2 * HID], gf_p[:])

                # embed paths (algo: 3 embeds of 32; sched: 4 embeds of 32)
                def embed_feats(dsrc, ncols, Atab, w1, w1_is_wb, w2w, w2b,
                                outslice, embdim):
                    ep = ppY.tile([B, ncols * embdim], FP32, tag="y")
                    for e in range(ncols):
                        ai = tl.tile([2, B], I32, tag="ai")
                        nc.sync.dma_start(
                            ai[:], bass.AP(dsrc, e, [[0, 2], [ncols, B]]))
                        af = tl.tile([2, B], FP32, tag="af")
                        nc.vector.tensor_copy(af[:], ai[:])
                        ot = tl.tile([2, B], FP32, tag="ot")
                        nc.vector.tensor_tensor(out=ot[:], in0=iop[:2, :],
                                                in1=af[:], op=ALU.is_equal)
                        mm(ep[:, e * embdim : (e + 1) * embdim], ot[:],
                           Atab[:, e * embdim : (e + 1) * embdim])
                    a_sb = tl.tile([B, ncols * embdim], FP32, tag="a_sb")
                    nc.vector.tensor_copy(a_sb[:], ep[:])
                    aT = transpose_to(tl, a_sb[:], B, ncols * embdim, extra=1,
                                      tag="embT")
                    f1 = ppY.tile([B, HID], FP32, tag="y")
                    if w1_is_wb:
                        mm(f1[:], aT[:], w1[:])
                    else:
                        mm(f1[:], aT[: ncols * embdim, :], w1[:], start=True,
                           stop=False)
                        mm(f1[:], ones_row[:, :B], w2b[:], start=False,
                           stop=True)  # unused path
                    f1r = tl.tile([B, HID], FP32, tag="embr")
                    nc.vector.tensor_scalar(out=f1r[:], in0=f1[:], scalar1=0.0,
                                            scalar2=None, op0=ALU.max)
                    f1T = transpose_to(tl, f1r[:], B, HID, tag="embT2")
                    f2 = ppY.tile([B, HID], FP32, tag="y")
                    mm(f2[:], f1T[:], w2w[:], start=True, stop=False)
                    mm(f2[:], ones_row[:, :B], w2b[:], start=False, stop=True)
                    nc.vector.tensor_copy(xcat[:, outslice], f2[:])

                embed_feats(d_algo, 3, par["A_algo"], par["algo1_wb"], True,
                            par["algo2_w"], par["algo2_b"],
                            slice(0, HID), DM)

                # sched1 takes [8,128] input: build s then 128+1 matmul
                ep = ppY.tile([B, HID], FP32, tag="y")
                for e in range(4):
                    ai = tl.tile([2, B], I32, tag="ai")
                    nc.sync.dma_start(ai[:],
                                      bass.AP(d_sched, e, [[0, 2], [4, B]]))
                    af = tl.tile([2, B], FP32, tag="af")
                    nc.vector.tensor_copy(af[:], ai[:])
                    ot = tl.tile([2, B], FP32, tag="ot")
                    nc.vector.tensor_tensor(out=ot[:], in0=iop[:2, :],
                                            in1=af[:], op=ALU.is_equal)
                    mm(ep[:, e * DM : (e + 1) * DM], ot[:],
                       par["A_sched"][:, e * DM : (e + 1) * DM])
                s_sb = tl.tile([B, HID], FP32, tag="a_sb")
                nc.vector.tensor_copy(s_sb[:], ep[:])
                sT = transpose_to(tl, s_sb[:], B, HID, tag="sT")
                sf1 = ppY.tile([B, HID], FP32, tag="y")
                mm(sf1[:], sT[:], par["sched1_w"][:], start=True, stop=False)
                mm(sf1[:], ones_row[:, :B], par["sched1_b"][:], start=False,
                   stop=True)
                sf1r = tl.tile([B, HID], FP32, tag="embr")
                nc.vector.tensor_scalar(out=sf1r[:], in0=sf1[:], scalar1=0.0,
                                        scalar2=None, op0=ALU.max)
                sf1T = transpose_to(tl, sf1r[:], B, HID, tag="embT2")
                sf2 = ppY.tile([B, HID], FP32, tag="y")
                mm(sf2[:], sf1T[:], par["sched2_w"][:], start=True, stop=False)
                mm(sf2[:], ones_row[:, :B], par["sched2_b"][:], start=False,
                   stop=True)
                nc.vector.tensor_copy(xcat[:, 2 * HID : 3 * HID], sf2[:])

                if dbg:
                    nc.sync.dma_start(d_dbg_poolf[:], poolF[:])
                    nc.sync.dma_start(d_dbg_xcat[:], xcat[:])
                # tokens [96, 32] (via DRAM bounce: partition regroup)
                with tc.tile_pool(name="xbounce", bufs=1, space="DRAM") as xb:
                    xcat_d = xb.tile([B, 3 * HID], FP32)
                    nc.sync.dma_start(xcat_d[:], xcat[:])
                    xtok = tl.tile([TOK, DM], FP32, tag="xtok")
                    nc.sync.dma_start(
                        xtok[:],
                        xcat_d[:].rearrange("b (t f) -> (b t) f", f=DM))

                SCL = float(1.0 / np.sqrt(HD))
                for l in range(4):
                    xT = transpose_to(tl, xtok[:], TOK, DM, extra=1,
                                      tag="xT")  # [33, 96]
                    qp = ppY.tile([HD, NH * TOK], FP32, tag="y")
                    kp = ppY.tile([HD, NH * TOK], FP32, tag="y")
                    for h in range(NH):
                        mm(qp[:, h * TOK : (h + 1) * TOK],
                           par[f"tf{l}_q"][:, h * HD : (h + 1) * HD], xT[:])
                        mm(kp[:, h * TOK : (h + 1) * TOK],
                           par[f"tf{l}_k"][:, h * HD : (h + 1) * HD], xT[:])
                    q_sb = tl.tile([HD, NH * TOK], FP32, tag="q_sb")
                    nc.vector.tensor_copy(q_sb[:], qp[:])
                    k_sb = tl.tile([HD, NH * TOK], FP32, tag="k_sb")
                    nc.vector.tensor_copy(k_sb[:], kp[:])
                    vp = ppY.tile([TOK, DM], FP32, tag="y")
                    mm(vp[:], xT[:], par[f"tf{l}_v"][:])
                    v_sb = tl.tile([TOK, DM], FP32, tag="v_sb")
                    nc.vector.tensor_copy(v_sb[:], vp[:])

                    sp = ppX.tile([TOK, NH * TOK], FP32, tag="x")
                    for h in range(NH):
                        mm(sp[:, h * TOK : (h + 1) * TOK],
                           q_sb[:, h * TOK : (h + 1) * TOK],
                           k_sb[:, h * TOK : (h + 1) * TOK])
                    mx = tl.tile([TOK, NH], FP32, tag="mx")
                    nc.vector.tensor_reduce(
                        out=mx[:], in_=sp[:].rearrange("p (h t) -> p h t",
                                                       h=NH),
                        axis=mybir.AxisListType.X, op=ALU.max)
                    nb = tl.tile([TOK, NH], FP32, tag="nb")
                    nc.vector.tensor_scalar(out=nb[:], in0=mx[:],
                                            scalar1=-SCL, scalar2=None,
                                            op0=ALU.mult)
                    P = tl.tile([TOK, NH * TOK], FP32, tag="P")
                    for h in range(NH):
                        nc.scalar.activation(P[:, h * TOK : (h + 1) * TOK],
                                             sp[:, h * TOK : (h + 1) * TOK],
                                             AF.Exp, bias=nb[:, h : h + 1],
                                             scale=SCL)
                    P2 = tl.tile([TOK, NH * TOK], FP32, tag="P2")
                    for h in range(NH):
                        nc.vector.tensor_tensor(
                            out=P2[:, h * TOK : (h + 1) * TOK],
                            in0=P[:, h * TOK : (h + 1) * TOK], in1=gmask[:],
                            op=ALU.mult)
                    rs = tl.tile([TOK, NH], FP32, tag="rs")
                    nc.vector.tensor_reduce(
                        out=rs[:], in_=P2[:].rearrange("p (h t) -> p h t",
                                                       h=NH),
                        axis=mybir.AxisListType.X, op=ALU.add)
                    rr = tl.tile([TOK, NH], FP32, tag="rr")
                    nc.vector.reciprocal(rr[:], rs[:])
                    P3 = tl.tile([TOK, NH * TOK], FP32, tag="P3")
                    for h in range(NH):
                        nc.vector.tensor_scalar(
                            out=P3[:, h * TOK : (h + 1) * TOK],
                            in0=P2[:, h * TOK : (h + 1) * TOK],
                            scalar1=rr[:, h : h + 1], scalar2=None,
                            op0=ALU.mult)
                    op_ = ppY.tile([TOK, DM], FP32, tag="y")
                    for h in range(NH):
                        ptp = ppX.tile([TOK, TOK], FP32, tag="x")
                        nc.tensor.transpose(
                            ptp[:], P3[:, h * TOK : (h + 1) * TOK],
                            idm[:TOK, :TOK])
                        pts = tl.tile([TOK, TOK], FP32, tag="pts")
                        nc.vector.tensor_copy(pts[:], ptp[:])
                        mm(op_[:, h * HD : (h + 1) * HD], pts[:],
                           v_sb[:, h * HD : (h + 1) * HD])
                    o_sb = tl.tile([TOK, DM], FP32, tag="o_sb")
                    nc.vector.tensor_copy(o_sb[:], op_[:])
                    oT = transpose_to(tl, o_sb[:], TOK, DM, extra=1, tag="oT")
                    prj = ppY.tile([TOK, DM], FP32, tag="y")
                    mm(prj[:], oT[:], par[f"tf{l}_o"][:])
                    x1 = tl.tile([TOK, DM], FP32, tag="x1")
                    nc.vector.tensor_tensor(out=x1[:], in0=xtok[:], in1=prj[:],
                                            op=ALU.add)

                    def layernorm(xin, gname, bname, tag):
                        mu = tl.tile([TOK, 1], FP32, tag=f"mu{tag}")
                        nc.vector.tensor_reduce(out=mu[:], in_=xin[:],
                                                axis=mybir.AxisListType.X,
                                                op=ALU.add)
                        nc.vector.tensor_scalar(out=mu[:], in0=mu[:],
                                                scalar1=1.0 / DM,
                                                scalar2=None, op0=ALU.mult)
                        xc = tl.tile([TOK, DM], FP32, tag=f"xc{tag}")
                        nc.vector.tensor_scalar(out=xc[:], in0=xin[:],
                                                scalar1=mu[:, 0:1],
                                                scalar2=None, op0=ALU.subtract)
                        sq = tl.tile([TOK, DM], FP32, tag=f"sq{tag}")
                        nc.vector.tensor_tensor(out=sq[:], in0=xc[:],
                                                in1=xc[:], op=ALU.mult)
                        vr = tl.tile([TOK, 1], FP32, tag=f"vr{tag}")
                        nc.vector.tensor_reduce(out=vr[:], in_=sq[:],
                                                axis=mybir.AxisListType.X,
                                                op=ALU.add)
                        sd = tl.tile([TOK, 1], FP32, tag=f"sd{tag}")
                        nc.scalar.activation(sd[:], vr[:], AF.Sqrt,
                                             bias=eps_col[:TOK, 0:1],
                                             scale=1.0 / DM)
                        rsd = tl.tile([TOK, 1], FP32, tag=f"rsd{tag}")
                        nc.vector.reciprocal(rsd[:], sd[:])
                        xn = tl.tile([TOK, DM], FP32, tag=f"xn{tag}")
                        nc.vector.tensor_scalar(out=xn[:], in0=xc[:],
                                                scalar1=rsd[:, 0:1],
                                                scalar2=None, op0=ALU.mult)
                        xg = tl.tile([TOK, DM], FP32, tag=f"xg{tag}")
                        nc.vector.tensor_tensor(out=xg[:], in0=xn[:],
                                                in1=par[gname][:], op=ALU.mult)
                        xo = tl.tile([TOK, DM], FP32, tag=f"xo{tag}")
                        nc.vector.tensor_tensor(out=xo[:], in0=xg[:],
                                                in1=par[bname][:], op=ALU.add)
                        return xo

                    x2 = layernorm(x1, f"tf{l}_ln1g", f"tf{l}_ln1b", "a")
                    xT2 = transpose_to(tl, x2[:], TOK, DM, extra=1, tag="xT2")
                    f1p = ppX.tile([TOK, FF], FP32, tag="x")
                    mm(f1p[:], xT2[:], par[f"tf{l}_ff1"][:])
                    f1r = tl.tile([TOK, FF], FP32, tag="f1r")
                    nc.vector.tensor_scalar(out=f1r[:], in0=f1p[:],
                                            scalar1=0.0, scalar2=None,
                                            op0=ALU.max)
                    f2p = ppY.tile([TOK, DM], FP32, tag="y")
                    for h2 in range(2):
                        fT = transpose_to(tl, f1r[:, h2 * 128 : (h2 + 1) * 128],
                                          TOK, 128, tag="fT")
                        mm(f2p[:], fT[:], par[f"tf{l}_ff2w"][:, h2, :],
                           start=(h2 == 0), stop=False)
                    mm(f2p[:], ones_row[:, :TOK], par[f"tf{l}_ff2b"][:],
                       start=False, stop=True)
                    x3 = tl.tile([TOK, DM], FP32, tag="x3")
                    nc.vector.tensor_tensor(out=x3[:], in0=x2[:], in1=f2p[:],
                                            op=ALU.add)
                    xtok = tl.tile([TOK, DM], FP32, tag="xtok")
                    xln = layernorm(x3, f"tf{l}_ln2g", f"tf{l}_ln2b", "b")
                    nc.vector.tensor_copy(xtok[:], xln[:])

                # head MLP
                xfin = tl.tile([B, 384], FP32)
                with tc.tile_pool(name="xbounce2", bufs=1, space="DRAM") as xb2:
                    xtok_d = xb2.tile([TOK, DM], FP32)
                    nc.sync.dma_start(xtok_d[:], xtok[:])
                    nc.sync.dma_start(
                        xfin[:],
                        xtok_d[:].rearrange("(b t) f -> b (t f)", b=B))
                m1p = ppY.tile([B, HID], FP32, tag="y")
                for h3 in range(3):
                    xfT = transpose_to(tl, xfin[:, h3 * 128 : (h3 + 1) * 128],
                                       B, 128, tag="xfT")
                    mm(m1p[:], xfT[:], par["mlp1_w"][:, h3, :],
                       start=(h3 == 0), stop=False)
                mm(m1p[:], ones_row[:, :B], par["mlp1_b"][:], start=False,
                   stop=True)
                m1r = tl.tile([B, HID], FP32)
                nc.vector.tensor_scalar(out=m1r[:], in0=m1p[:], scalar1=0.0,
                                        scalar2=None, op0=ALU.max)
                m1T = transpose_to(tl, m1r[:], B, HID, tag="m1T")
                m2p = ppY.tile([B, 64], FP32, tag="y")
                mm(m2p[:], m1T[:], par["mlp2_w"][:], start=True, stop=False)
                mm(m2p[:], ones_row[:, :B], par["mlp2_b"][:], start=False,
                   stop=True)
                m2r = tl.tile([B, 64], FP32)
                nc.vector.tensor_scalar(out=m2r[:], in0=m2p[:], scalar1=0.0,
                                        scalar2=None, op0=ALU.max)
                m2T = transpose_to(tl, m2r[:], B, 64, extra=1, tag="m2T")
                m3p = ppY.tile([B, 1], FP32, tag="y")
                mm(m3p[:], m2T[:], par["mlp3_wb"][:])
                res = tl.tile([B, 1], FP32)
                nc.vector.tensor_copy(res[:], m3p[:])
                nc.sync.dma_start(d_out[:], res[:])

    nc.compile()

    input_specs = {}
    for name, h in ins.items():
        input_specs[name] = name
    return nc


# ----------------------------------------------------------------------------
# Runner
# ----------------------------------------------------------------------------

_CACHE = {}
_PREP_CACHE = {}


def make_in_maps(cfg, rep, per_core, hp, algo_ops, schedule):
    algo = np.ascontiguousarray(np.asarray(algo_ops, dtype=np.int32))
    sched = np.ascontiguousarray(np.asarray(schedule, dtype=np.int32))
    in_maps = []
    for c in range(NCORES):
        m = dict(rsD=rep["rsD"], reD=rep["reD"], rsS=rep["rsS"],
                 reS=rep["reS"], gid=rep["gid"], algo=algo, sched=sched)
        m.update(per_core[c])
        m.update(hp)
        in_maps.append(m)
    return in_maps


def kernel(algo_ops, schedule, edge_src, edge_dst, node2graph, params,
           n_true=None):
    from concourse.bass_utils import run_bass_kernel_spmd

    import hashlib

    edge_src = np.ascontiguousarray(np.asarray(edge_src, dtype=np.int32))
    edge_dst = np.ascontiguousarray(np.asarray(edge_dst, dtype=np.int32))
    node2graph = np.ascontiguousarray(np.asarray(node2graph, dtype=np.int32))
    if n_true is None:
        n_true = int(node2graph.shape[0])
    E = int(edge_src.shape[0])
    h = hashlib.blake2b(digest_size=16)
    h.update(edge_src.tobytes())
    h.update(edge_dst.tobytes())
    h.update(node2graph.tobytes())
    pkey = (n_true, E, h.hexdigest())
    if pkey in _PREP_CACHE:
        cfg, rep, per_core = _PREP_CACHE[pkey]
    else:
        cfg, rep, per_core = host_prep(edge_src, edge_dst, node2graph,
                                       n_true, E)
        _PREP_CACHE.clear()
        _PREP_CACHE[pkey] = (cfg, rep, per_core)
    hp = host_params(params)

    edge_fp16 = os.environ.get("GNN_EDGE_FP16", "0") == "1"
    key = (cfg.NT, cfg.E, tuple(cfg.KW), edge_fp16)
    if key not in _CACHE:
        _CACHE[key] = build_program(cfg, edge_fp16=edge_fp16)
    nc = _CACHE[key]

    in_maps = make_in_maps(cfg, rep, per_core, hp, algo_ops, schedule)
    trace = os.environ.get("GNN_TRACE", "0") == "1"
    if trace:
        try:
            import antenv.axon_hooks  # noqa: F401  (NTFF hook availability)
        except ImportError:
            trace = False
    res = run_bass_kernel_spmd(nc, in_maps, core_ids=list(range(NCORES)),
                               trace=trace)
    out = res.results[0]["out"]
    if trace and res.exec_time_ns is not None:
        print(f"HW exec time: {res.exec_time_ns} ns")
    return np.asarray(out, dtype=np.float32)
